# revision 18
# baseline (speedup 1.0000x reference)
"""BiMamba (bimamba_type='v2') Trainium2 Bass kernel.

Data-parallel over the fused B*N=828 (padded to 896) sequence axis across 8
NeuronCores (112 sequences/core, 8 chunks of 14). Key design points:
  - SCAN4_ANT: custom DVE op (hand-built uOp tables, registered at runtime
    into the ant custom-op rows) runs the selective scan as four
    interleaved recurrences with states in the block-1/3 a/b result flops;
    the 2x_2p perf slot processes packed bf16 pairs at 2 elem/cycle —
    ~3.9x the stock tensor_tensor_scan (which pays a feedback bubble).
    Scan tensors live in a 4-chain layout [p, n, b4, t, bpair, branch]
    (chain = seq-pair half x branch), produced interleaved at the source.
  - depthwise causal conv folded into PE: per tap k, matmul of
    w_in_x[c,d]*conv_w[d,k] against shifted windows of the zero-padded LN1
    output (bwd branch via a reversed padded copy); front-end in bf16.
  - act-table patch: Exp/Ln resolve to natural_log_exp_and_others, killing
    the per-switch ACT_TABLE_LOAD ping-pong; PSUM->SBUF copies on ACT.
  - explicit front/back software pipelining (front(ch+1) emitted before
    back(ch)) with parity-buffered dA/brep tiles.
  - dt = ln(1+exp(.)) (no softplus table); LN rstd = exp(-0.5*ln(var+eps)).
"""

import numpy as np
import ml_dtypes

import concourse.bass as bass
import concourse.tile as tile
from concourse import bacc, mybir
from concourse.bass_utils import run_bass_kernel_spmd

# --- SCAN4_ANT: custom DVE op — 4-interleaved-chain multiply-add scan.
# Stream elements rotate over four independent recurrences (chain = k mod 4):
#   s[c] = d0[k]*s[c] + d1[k]; out[k] = s[c]
# States live in blocks 1/3's a/b result flops. The 1x slot issues 1
# elem/cycle (state re-read 4 cycles after write); the 2x_2p slot processes
# packed bf16 pairs at 2 elems/cycle, pairs alternating between chain groups
# (0,1) and (2,3) so each group's state is re-read 2 cycles after writing.
from dataclasses import dataclass as _dataclass

from concourse import dve_ops as _ops_mod
from concourse.dve_ops import _COMPILE_CACHE as _DVE_CACHE
from concourse.dve_spec import Spec as _Spec, Src0 as _Src0, Src1 as _Src1
from concourse.dve_uop import (
    ENABLE as _EN,
    AluInp as _AluInp,
    AluOp as _AluOp,
    DelayInp as _DelayInp,
    DveOpSpec as _DveOpSpec,
    InpSel as _InpSel,
    OutPath as _OutPath,
    OutSel as _OutSel,
    Trigger as _Trigger,
    UopConfig as _UopConfig,
)

_SCAN_NAME = "SCAN4_ANT"
_SCAN_ROW = 17  # rows 1..16 used by stock OPS; byte-36 row field < 0x20
_SEGDOT_NAME = "SEGDOT8_ANT"
_SEGDOT_ROW = 18


def _uop_1x(chain, init, nxt):
    u = _UopConfig()
    u.enable_input(_InpSel.SRC_0, 0)
    u.enable_input(_InpSel.SRC_1, 1)
    if init:
        u.enable_input(_InpSel.ZERO, 2)
    u.require_inp0 = _EN
    u.require_inp1 = _EN
    u.repeat_count = 1
    u.trigger = (_Trigger.SRC_TENSOR_DONE, _Trigger.COUNT, _Trigger.NONE)
    u.next_uop = (0, nxt, 0)
    u.enable_output(_OutSel.ALU_OUT, _OutPath.WR0_LO)
    mb, ab = (0, 1) if chain < 2 else (2, 3)
    flop_a = chain % 2 == 0
    state_src = _AluInp.PREV_DELAY_1 if init else (
        _AluInp.NEXT_ALU_OUT_A if flop_a else _AluInp.NEXT_ALU_OUT_B)
    for k in range(0, mb):
        u.datapath_config[k].pass_through_alu()
        u.datapath_config[k].pass_through_delay(0)
        if init:
            u.datapath_config[k].pass_through_delay(1)
    u.datapath_config[mb].enable_alu(_AluOp.MULTIPLY, _AluInp.PREV_ALU_OUT,
                                     state_src)
    u.datapath_config[mb].pass_through_delay(0)
    u.datapath_config[ab].enable_alu(_AluOp.ADD, _AluInp.PREV_ALU_OUT,
                                     _AluInp.PREV_DELAY_0)
    if flop_a:
        u.datapath_config[ab].alu_out_a_enable = _EN
    else:
        u.datapath_config[ab].alu_out_b_enable = _EN
    for k in range(ab + 1, 8):
        u.datapath_config[k].pass_through_alu()
    return u


def _uop_2x(group, init, nxt):
    u = _UopConfig()
    u.enable_input(_InpSel.SRC_0, 0)
    u.enable_input(_InpSel.SRC_1, 1)
    u.enable_input(_InpSel.SRC_0_HI, 2)
    u.enable_input(_InpSel.SRC_1_HI, 3)
    if init:
        u.enable_input(_InpSel.ZERO, 4)
    u.require_inp0 = _EN
    u.require_inp1 = _EN
    u.repeat_count = 1
    u.trigger = (_Trigger.SRC_TENSOR_DONE, _Trigger.COUNT, _Trigger.NONE)
    u.next_uop = (0, nxt, 0)
    u.enable_output(_OutSel.DELAY_3, _OutPath.WR0_LO)
    u.enable_output(_OutSel.ALU_OUT, _OutPath.WR0_HI)
    flop_a = group == 0
    st = _AluInp.PREV_DELAY_3 if init else (
        _AluInp.NEXT_ALU_OUT_A if flop_a else _AluInp.NEXT_ALU_OUT_B)
    b0 = u.datapath_config[0]
    b0.enable_alu(_AluOp.MULTIPLY, _AluInp.PREV_ALU_OUT, st)
    b0.pass_through_delay(0, 1, 2)
    if init:
        b0.pass_through_delay(3)
    b1 = u.datapath_config[1]
    b1.enable_alu(_AluOp.ADD, _AluInp.PREV_ALU_OUT, _AluInp.PREV_DELAY_0)
    if flop_a:
        b1.alu_out_a_enable = _EN
    else:
        b1.alu_out_b_enable = _EN
    b1.pass_through_delay(1, 2)
    if init:
        b1.pass_through_delay(3)
    b2 = u.datapath_config[2]
    b2.enable_alu(_AluOp.MULTIPLY, _AluInp.PREV_DELAY_1, st)
    b2.pass_through_delay(2)
    b2.enable_delay_from_src(_DelayInp.PREV_ALU_OUT, 3)
    b3 = u.datapath_config[3]
    b3.enable_alu(_AluOp.ADD, _AluInp.PREV_ALU_OUT, _AluInp.PREV_DELAY_2)
    if flop_a:
        b3.alu_out_a_enable = _EN
    else:
        b3.alu_out_b_enable = _EN
    b3.pass_through_delay(3)
    for k in range(4, 8):
        u.datapath_config[k].pass_through_alu()
        u.datapath_config[k].pass_through_delay(3)
    return u


# --- SEGDOT8_ANT: segmented dot-product. Stream = segments of 32 elements
# ([n=8 outer] x [quad=4 inner]); the 4 quad items are independent
# accumulators (chains); output = 4 values per segment:
#   out[seg, q] = sum_n in0[seg, n, q] * in1[seg, n, q]
# In 2x mode each cycle processes a packed quad-pair; groups (q0,q1)/(q2,q3)
# alternate. acc_lo lives in block-2 a/b flops (read by block-1's ADD via
# NEXT_ALU_OUT), acc_hi in block-5 a/b flops.


def _sd_uop_2x(group, init, emit, nxt):
    u = _UopConfig()
    u.enable_input(_InpSel.SRC_0, 0)
    u.enable_input(_InpSel.SRC_1, 1)
    u.enable_input(_InpSel.SRC_0_HI, 2)
    u.enable_input(_InpSel.SRC_1_HI, 3)
    if init:
        u.enable_input(_InpSel.ZERO, 4)
    u.require_inp0 = _EN
    u.require_inp1 = _EN
    u.repeat_count = 1
    u.trigger = (_Trigger.SRC_TENSOR_DONE, _Trigger.COUNT, _Trigger.NONE)
    u.next_uop = (0, nxt, 0)
    if emit:
        u.enable_output(_OutSel.DELAY_3, _OutPath.WR0_LO)
        u.enable_output(_OutSel.ALU_OUT, _OutPath.WR0_HI)
    flop_a = group == 0
    st = _AluInp.PREV_DELAY_3 if init else _AluInp.NEXT_ALU_OUT_A
    if not init and not flop_a:
        st = _AluInp.NEXT_ALU_OUT_B
    # b0: m_lo = h_lo * c_lo   (delay0 = c_lo consumed here)
    b0 = u.datapath_config[0]
    b0.enable_alu(_AluOp.MULTIPLY, _AluInp.PREV_ALU_OUT, _AluInp.PREV_DELAY_0)
    b0.pass_through_delay(1, 2)
    if init:
        b0.pass_through_delay(3)
    # b1: acc_lo' = m_lo + acc_lo (b2's a/b flop; ZERO via delay3 on init)
    b1 = u.datapath_config[1]
    b1.enable_alu(_AluOp.ADD, _AluInp.PREV_ALU_OUT, st)
    b1.pass_through_delay(1, 2)
    if init:
        b1.pass_through_delay(3)
    # b2: bypass acc_lo' into b2's a/b flop
    b2 = u.datapath_config[2]
    b2.pass_through_alu()
    if flop_a:
        b2.alu_out_a_enable = _EN
    else:
        b2.alu_out_b_enable = _EN
    b2.pass_through_delay(1, 2)
    if init:
        b2.pass_through_delay(3)
    # b3: m_hi = h_hi * c_hi; on emit also stage acc_lo' into delay3
    b3 = u.datapath_config[3]
    b3.enable_alu(_AluOp.MULTIPLY, _AluInp.PREV_DELAY_1, _AluInp.PREV_DELAY_2)
    if emit:
        b3.enable_delay_from_src(_DelayInp.PREV_ALU_OUT, 3)
    elif init:
        b3.pass_through_delay(3)
    # b4: acc_hi' = m_hi + acc_hi (b5's a/b flop)
    b4 = u.datapath_config[4]
    st_hi = _AluInp.PREV_DELAY_3 if init else _AluInp.NEXT_ALU_OUT_A
    if not init and not flop_a:
        st_hi = _AluInp.NEXT_ALU_OUT_B
    b4.enable_alu(_AluOp.ADD, _AluInp.PREV_ALU_OUT, st_hi)
    if emit:
        b4.pass_through_delay(3)
    # b5: bypass acc_hi' into b5's a/b flop
    b5 = u.datapath_config[5]
    b5.pass_through_alu()
    if flop_a:
        b5.alu_out_a_enable = _EN
    else:
        b5.alu_out_b_enable = _EN
    if emit:
        b5.pass_through_delay(3)
    for k in range(6, 8):
        u.datapath_config[k].pass_through_alu()
        if emit:
            u.datapath_config[k].pass_through_delay(3)
    return u


def _sd_uop_1x(chain, init, emit, nxt):
    u = _UopConfig()
    u.enable_input(_InpSel.SRC_0, 0)
    u.enable_input(_InpSel.SRC_1, 1)
    if init:
        u.enable_input(_InpSel.ZERO, 2)
    u.require_inp0 = _EN
    u.require_inp1 = _EN
    u.repeat_count = 1
    u.trigger = (_Trigger.SRC_TENSOR_DONE, _Trigger.COUNT, _Trigger.NONE)
    u.next_uop = (0, nxt, 0)
    if emit:
        u.enable_output(_OutSel.ALU_OUT, _OutPath.WR0_LO)
    flop_a = chain % 2 == 0
    ab = 1 if chain < 2 else 3         # ADD block; store block = ab+1
    st = _AluInp.PREV_DELAY_1 if init else (
        _AluInp.NEXT_ALU_OUT_A if flop_a else _AluInp.NEXT_ALU_OUT_B)
    b0 = u.datapath_config[0]
    b0.enable_alu(_AluOp.MULTIPLY, _AluInp.PREV_ALU_OUT, _AluInp.PREV_DELAY_0)
    if init:
        b0.pass_through_delay(1)
    for k in range(1, ab):
        u.datapath_config[k].pass_through_alu()
        if init:
            u.datapath_config[k].pass_through_delay(1)
    u.datapath_config[ab].enable_alu(_AluOp.ADD, _AluInp.PREV_ALU_OUT, st)
    bs = u.datapath_config[ab + 1]
    bs.pass_through_alu()
    if flop_a:
        bs.alu_out_a_enable = _EN
    else:
        bs.alu_out_b_enable = _EN
    for k in range(ab + 2, 8):
        u.datapath_config[k].pass_through_alu()
    return u


@_dataclass(frozen=True)
class _ShimSpec:
    accum: object = None


class _ScanOp:
    name = _SCAN_NAME
    subdim = False
    spec = _ShimSpec()
    perf_en: dict = {}

    def compile(self, ver):
        key = (self.name, ver)
        if key not in _DVE_CACHE:
            uops = [
                _uop_1x(0, True, 1), _uop_1x(1, True, 2),
                _uop_1x(2, True, 3), _uop_1x(3, True, 4),
                _uop_1x(0, False, 5), _uop_1x(1, False, 6),
                _uop_1x(2, False, 7), _uop_1x(3, False, 4),
            ]
            u2 = [
                _uop_2x(0, True, 1), _uop_2x(1, True, 2),
                _uop_2x(0, False, 3), _uop_2x(1, False, 2),
                _uop_2x(0, False, 3), _uop_2x(1, False, 2),
                _uop_2x(0, False, 3), _uop_2x(1, False, 2),
            ]
            u2p = [
                _uop_2x(0, True, 1), _uop_2x(1, True, 2),
                _uop_2x(0, False, 3), _uop_2x(1, False, 2),
                _uop_2x(0, False, 3), _uop_2x(1, False, 2),
                _uop_2x(0, False, 3), _uop_2x(1, False, 2),
            ]
            _DVE_CACHE[key] = _DveOpSpec(
                name=self.name, opcode=_SCAN_ROW, uops=uops,
                uops_2x=u2, uops_2x_2p=u2p, perf_max=2, rd1_en=True)
        return _DVE_CACHE[key]


_SCAN4 = _ScanOp()


class _SegDotOp:
    name = _SEGDOT_NAME
    subdim = False
    spec = _ShimSpec()
    perf_en: dict = {}

    def compile(self, ver):
        key = (self.name, ver)
        if key not in _DVE_CACHE:
            def chain2x():
                us = [_sd_uop_2x(0, True, False, 2)]     # entry
                us.append(_sd_uop_2x(0, True, False, 2))  # loop initA
                us.append(_sd_uop_2x(1, True, False, 3))  # initB
                for j in range(6):
                    us.append(_sd_uop_2x(0, False, False, 4 + 2 * j))
                    us.append(_sd_uop_2x(1, False, False, 5 + 2 * j))
                us.append(_sd_uop_2x(0, False, True, 16))
                us.append(_sd_uop_2x(1, False, True, 1))
                return us

            # The op's APs statically satisfy the 2X_1PORT trigger
            # conditions (2B dtype, unit inner step, 4B alignment), so the
            # REGULAR slot is never reached — fill it with the 2x chain to
            # stay inside the 256-entry control table. uops_2x_2p=None
            # reuses the 2X_1PORT continuation slots.
            _DVE_CACHE[key] = _DveOpSpec(
                name=self.name, opcode=_SEGDOT_ROW, uops=chain2x(),
                uops_2x=chain2x(), uops_2x_2p=None, perf_max=2,
                rd1_en=True)
        return _DVE_CACHE[key]


_SEGDOT = _SegDotOp()


def _segdot_register():
    if _SEGDOT_NAME in _ops_mod._SUB_OPCODE_FOR_NAME:
        return
    _ops_mod._SUB_OPCODE_FOR_NAME[_SEGDOT_NAME] = _SEGDOT_ROW
    _ops_mod.OPS.append(_SEGDOT)
    _ops_mod.CUSTOM_DVE_SPECS[_SEGDOT_NAME] = _Spec(
        body=_Src0 * _Src1,
        reference=lambda in0, in1, s0, s1, imm2: in0 * in1,
    )


def _segdot_emit(nc, out, in0, in1):
    _segdot_register()
    from concourse import bass_isa
    from concourse.bass_utils import dve_ver_for

    v = nc.vector
    if _SEGDOT.name not in v.bass.m.ant_custom_dve_ops:
        v.bass.m.ant_custom_dve_ops = sorted(
            {*v.bass.m.ant_custom_dve_ops, _SEGDOT.name})
    _SEGDOT.compile(dve_ver_for(v.bass.trn_type))
    shape = bass_isa.CustomDveShape.TTSS
    isa_opcode = v.bass.isa.Opcode[
        f"NEURON_ISA_TPB_OPCODE_CUSTOM_DVE_ANT_{shape.slot()}"].value
    imm = mybir.ImmediateValue(dtype=mybir.dt.float32, value=0.0)
    inst = bass_isa.InstCustomDveAnt(
        name=v.bass.get_next_instruction_name(),
        op_name=_SEGDOT.name,
        rd1_en=True,
        subdim=0,
        imm2=0.0,
        shape=shape,
        row=_SEGDOT_ROW,
        isa_opcode=isa_opcode,
        ins=[v.lower_ap(in0, for_isa=True),
             v.lower_ap(in1, for_isa=True), imm, imm],
        outs=[v.lower_ap(out, for_isa=True)],
    )
    inst.perf_max = 2
    return v.add_instruction(inst)


def _scan4_register():
    if _SCAN_NAME in _ops_mod._SUB_OPCODE_FOR_NAME:
        return
    _ops_mod._SUB_OPCODE_FOR_NAME[_SCAN_NAME] = _SCAN_ROW
    _ops_mod.OPS.append(_SCAN4)
    _ops_mod.CUSTOM_DVE_SPECS[_SCAN_NAME] = _Spec(
        body=_Src0 * _Src1,
        reference=lambda in0, in1, s0, s1, imm2: in0 * in1,
    )


def _scan4_emit(nc, out, d0, d1):
    _scan4_register()
    from concourse import bass_isa
    from concourse.bass_utils import dve_ver_for

    v = nc.vector
    if _SCAN4.name not in v.bass.m.ant_custom_dve_ops:
        v.bass.m.ant_custom_dve_ops = sorted(
            {*v.bass.m.ant_custom_dve_ops, _SCAN4.name})
    _SCAN4.compile(dve_ver_for(v.bass.trn_type))
    shape = bass_isa.CustomDveShape.TTSS
    isa_opcode = v.bass.isa.Opcode[
        f"NEURON_ISA_TPB_OPCODE_CUSTOM_DVE_ANT_{shape.slot()}"].value
    imm = mybir.ImmediateValue(dtype=mybir.dt.float32, value=0.0)
    inst = bass_isa.InstCustomDveAnt(
        name=v.bass.get_next_instruction_name(),
        op_name=_SCAN4.name,
        rd1_en=True,
        subdim=0,
        imm2=0.0,
        shape=shape,
        row=_SCAN_ROW,
        isa_opcode=isa_opcode,
        ins=[v.lower_ap(d0, for_isa=True),
             v.lower_ap(d1, for_isa=True), imm, imm],
        outs=[v.lower_ap(out, for_isa=True)],
    )
    inst.perf_max = 2
    return v.add_instruction(inst)



F32 = mybir.dt.float32
BF16 = mybir.dt.bfloat16
AF = mybir.ActivationFunctionType
ALU = mybir.AluOpType

B, T, N, C = 4, 24, 207, 128
DI = 256
DS = 16
RK = 8
EPS = 1e-5
NCORES = 8
BSEQ = 832
BC = BSEQ // NCORES          # 104 sequences per core (828 real + 4 pad)
NCHUNK = 8
CBS = (14, 14, 14, 14, 14, 14, 10, 10)   # per-chunk seqs (must be even)
OFF = tuple(sum(CBS[:i]) for i in range(NCHUNK))
CB = max(CBS)                # tile-shape maximum
B4 = CB // 2                 # sequence pairs (scan chain interleave)
CBT = CB * T                 # tokens per max chunk
TP = T + 3                   # left-padded time for causal conv windows

# --- act-table patch: make the set chooser pick natural_log_exp_and_others
# for both Exp and Ln (otherwise it alternates exp_and_others/natural_log
# and reloads tables on every switch).
import concourse.bacc as _bacc_mod
from concourse.hw_specs import get_activation_tables as _orig_gat


def _patched_gat(arch):
    t = dict(_orig_gat(arch))
    for nm, drop in (("exp_and_others", AF.Exp), ("exp_and_friends", AF.Exp),
                     ("natural_log", AF.Ln)):
        if nm in t:
            t[nm] = set(t[nm]) - {drop}
    return t


_bacc_mod.get_activation_tables = _patched_gat


def _pbcast(ap, parts=128):
    a = [[0, parts]] + [list(x) for x in ap.ap]
    return bass.AP(tensor=ap.tensor, offset=ap.offset, ap=a)


def _rev_t(ap):
    a = [list(x) for x in ap.ap]
    st, ct = a[-1]
    off = ap.offset + st * (ct - 1)
    a[-1] = [-st, ct]
    return bass.AP(tensor=ap.tensor, offset=off, ap=a)


def _zstride(ap, dim, count):
    a = [list(x) for x in ap.ap]
    a.insert(1 + dim, [0, count])
    return bass.AP(tensor=ap.tensor, offset=ap.offset, ap=a)


def _ap(base, dims, offset=0):
    """AP over base's tensor: keep base's partition dim, explicit free dims
    [[stride, count], ...], extra element offset."""
    return bass.AP(tensor=base.tensor, offset=base.offset + offset,
                   ap=[list(base.ap[0])] + [list(d) for d in dims])


def build_program(a_pow, ln_trivial=(False, False)):
    nc = bacc.Bacc("TRN2", target_bir_lowering=False, debug=False,
                   enable_asserts=False, num_devices=NCORES)

    def din(name, shape, dt=F32):
        return nc.dram_tensor(name, shape, dt, kind="ExternalInput").ap()

    xin = din("xin", [C, BC, T], BF16)
    w_z = din("w_z", [C, 2 * C], BF16)            # z half of in_proj
    wconv = din("wconv", [C, 2, 2, 4, C], BF16)   # [c, br, ti, k, d]
    convb = din("convb", [128, 2, 2, 1])
    xw = din("xw", [128, 2, 2, 40], BF16)
    dtw = din("dtw", [RK, 2, DI], BF16)
    dtb = din("dtb", [128, 2, 2, 1])
    dpc = din("dpc", [128, 2, 2, 1])
    wout = din("wout", [128, 2, C], BF16)
    ln1g = din("ln1g", [C, 1])
    ln1b = din("ln1b", [C, 1])
    ln2g = din("ln2g", [C, 1])
    ln2b = din("ln2b", [C, 1])
    out = nc.dram_tensor("out", [C, BC, T], F32, kind="ExternalOutput").ap()

    with tile.TileContext(nc) as tc, \
         tc.tile_pool(name="weights", bufs=1) as wp, \
         tc.tile_pool(name="small", bufs=2) as sp, \
         tc.tile_pool(name="stats", bufs=2) as stp, \
         tc.tile_pool(name="dbu", bufs=1) as bp, \
         tc.tile_pool(name="brep", bufs=2) as brp, \
         tc.tile_pool(name="crep", bufs=1) as crp, \
         tc.tile_pool(name="dram", bufs=2, space="DRAM") as drp, \
         tc.tile_pool(name="psA", bufs=2, space="PSUM") as psA, \
         tc.tile_pool(name="psCv", bufs=2, space="PSUM") as psCv, \
         tc.tile_pool(name="psB", bufs=2, space="PSUM") as psB, \
         tc.tile_pool(name="psO", bufs=2, space="PSUM") as psO:

        pre_u = {}
        for ch0 in range(2):
            t = sp.tile([C, CB, T], BF16, tag="u", name=f"u{ch0}", bufs=3)
            nc.sync.dma_start(t[:, 0:CBS[ch0], :],
                              xin[:, OFF[ch0]:OFF[ch0] + CBS[ch0], :])
            pre_u[ch0] = t

        def load_w(name, ap_src, shape, dt=F32):
            t = wp.tile(shape, dt, tag=name, name=name)
            nc.sync.dma_start(t[:], ap_src)
            return t

        w_z_sb = load_w("w_z", w_z, [C, 2 * C], BF16)
        # wconv is 512KB — split the load across 8 DMAs so it spreads over
        # parallel queues instead of serializing ~23us on one
        wconv_sb = wp.tile([C, 2, 2, 4, C], BF16, tag="wconv", name="wconv")
        for _br in range(2):
            for _ti in range(2):
                for _kh in range(2):
                    nc.sync.dma_start(
                        wconv_sb[:, _br, _ti, 2 * _kh:2 * _kh + 2, :],
                        wconv[:, _br, _ti, 2 * _kh:2 * _kh + 2, :])
        convb_sb = load_w("convb", convb, [128, 2, 2, 1])
        xw_sb = load_w("xw", xw, [128, 2, 2, 40], BF16)
        dtw_sb = load_w("dtw", dtw, [RK, 2, DI], BF16)
        dtb_sb = load_w("dtb", dtb, [128, 2, 2, 1])
        dpc_sb = load_w("dpc", dpc, [128, 2, 2, 1])
        wout_sb = load_w("wout", wout, [128, 2, C], BF16)
        ones_bf = wp.tile([C, 1], BF16, tag="ones_bf")
        nc.vector.memset(ones_bf[:], 1.0 / C)
        ln1g_sb = load_w("ln1g", ln1g, [C, 1])
        ln1b_sb = load_w("ln1b", ln1b, [C, 1])
        ln2g_sb = load_w("ln2g", ln2g, [C, 1])
        ln2b_sb = load_w("ln2b", ln2b, [C, 1])
        ones_sb = wp.tile([C, 1], F32, tag="ones")
        nc.vector.memset(ones_sb[:], 1.0 / C)
        eps_sb = wp.tile([C, 1], F32, tag="eps")
        nc.vector.memset(eps_sb[:], EPS)
        ones_row = wp.tile([1, C], F32, tag="ones_row")
        nc.vector.memset(ones_row[:], 1.0)

        # persistent padded LN1 outputs (fwd + reversed), 2 parities
        hlnp = [wp.tile([C, CB, TP], BF16, tag=f"hlnp{i}", name=f"hlnp{i}")
                for i in range(2)]
        hlnr = [wp.tile([C, CB, TP], BF16, tag=f"hlnr{i}", name=f"hlnr{i}")
                for i in range(2)]
        for tl in hlnp + hlnr:
            nc.gpsimd.memset(tl[:, :, 0:3], 0.0)

        # persistent dA tiles in 4-chain interleaved layout
        # [p, n, b4, t, bpair, a]: ti=0 double-buffered (exps in front),
        # ti=1 single (exps at back start). t=0 column zero = segment reset.
        dA0 = [wp.tile([128, DS, B4, T, 2, 2], BF16, tag=f"dA0_{i}",
                       name=f"dA0_{i}") for i in range(2)]
        dA1 = wp.tile([128, DS, B4, T, 2, 2], BF16, tag="dA1", name="dA1")
        for tl in dA0 + [dA1]:
            nc.gpsimd.memset(tl[:, :, :, 0:1, :, :], 0.0)

        def layernorm(src_f32, g_sb, b_sb, dst, trivial=False, cbt=CBT):
            """LN over channel (partition) dim of src [C, CBT] -> dst view."""
            sq = sp.tile([C, CBT], BF16, tag="ln_sq", bufs=1)
            nc.scalar.activation(sq[:, 0:cbt], src_f32, AF.Square)
            ps_s = psA.tile([128, CBT], F32, tag="pm", name="ps_s")
            ps_q = psA.tile([128, CBT], F32, tag="pm", name="ps_q")
            ones_like = (ones_bf if src_f32.tensor.dtype == BF16
                         else ones_sb)
            nc.tensor.matmul(ps_s[0:1, 0:cbt], ones_like[:], src_f32,
                             start=True, stop=True)
            nc.tensor.matmul(ps_q[0:1, 0:cbt], ones_bf[:], sq[:, 0:cbt],
                             start=True, stop=True)
            # ones vectors carry 1/C, so ps_s = mean, ps_q = E[x^2]
            mean = stp.tile([1, CBT], F32, tag="mean")
            nc.scalar.copy(mean[:, 0:cbt], ps_s[0:1, 0:cbt])
            var = stp.tile([1, CBT], F32, tag="var")
            m2 = stp.tile([1, CBT], F32, tag="m2")
            nc.scalar.square(m2[:, 0:cbt], ps_s[0:1, 0:cbt])
            nc.vector.tensor_sub(var[:, 0:cbt], ps_q[0:1, 0:cbt],
                                 m2[:, 0:cbt])
            # rstd = (var+eps)^-0.5 = exp(-0.5*ln(var+eps))
            nc.scalar.activation(var[:, 0:cbt], var[:, 0:cbt], AF.Ln,
                                 bias=eps_sb[0:1, 0:1])
            nc.scalar.activation(var[:, 0:cbt], var[:, 0:cbt], AF.Exp,
                                 scale=-0.5)
            mean_r = psB.tile([C, CBT], F32, tag="pb", name="mean_r")
            nc.tensor.matmul(mean_r[:, 0:cbt], ones_row[:], mean[:, 0:cbt],
                             start=True, stop=True)
            rstd_r = psB.tile([C, CBT], F32, tag="pb", name="rstd_r")
            nc.tensor.matmul(rstd_r[:, 0:cbt], ones_row[:], var[:, 0:cbt],
                             start=True, stop=True)
            tmp = sp.tile([C, CBT], BF16, tag="ln_tmp", bufs=1)
            nc.vector.tensor_sub(tmp[:, 0:cbt], src_f32, mean_r[:, 0:cbt])
            if trivial:
                # g==1, b==0: write the normalize directly to dst
                rv = rstd_r[:, 0:cbt]
                if len(dst.shape) == 3:
                    tv = tmp[:, 0:cbt].rearrange("p (b t) -> p b t",
                                                 t=dst.shape[2])
                    rv = rv.rearrange("p (b t) -> p b t", t=dst.shape[2])
                else:
                    tv = tmp[:, 0:cbt]
                nc.vector.tensor_mul(dst, tv, rv)
                return
            nc.vector.tensor_mul(tmp[:, 0:cbt], tmp[:, 0:cbt],
                                 rstd_r[:, 0:cbt])
            tv = tmp[:, 0:cbt]
            if len(dst.shape) == 3:
                tv = tv.rearrange("p (b t) -> p b t", t=dst.shape[2])
            nc.vector.tensor_scalar(dst, tv, g_sb[:, 0:1], b_sb[:, 0:1],
                                    ALU.mult, ALU.add)

        state = {}
        tails = {}

        def front(ch):
            par = ch % 2
            b0, cb = OFF[ch], CBS[ch]
            b4, cbt = cb // 2, cb * T
            if ch in pre_u:
                u = pre_u.pop(ch)
            else:
                u = sp.tile([C, CB, T], BF16, tag="u", name=f"u{ch}",
                            bufs=3)
                nc.sync.dma_start(u[:, 0:cb, :], xin[:, b0:b0 + cb, :])
            uf = u[:, 0:cb, :].rearrange("p b t -> p (b t)")

            hp, hr = hlnp[par], hlnr[par]
            layernorm(uf, ln1g_sb, ln1b_sb, hp[:, 0:cb, 3:TP],
                      trivial=ln_trivial[0], cbt=cbt)
            # reversed copy for the bwd-branch conv windows
            nc.scalar.copy(hr[:, 0:cb, 3:TP], _rev_t(hp[:, 0:cb, 3:TP]))

            # z half + silu gate
            sz = [sp.tile([128, B4, T, 2], BF16, tag=f"sz{ti}",
                          name=f"sz{ti}_{ch}") for ti in range(2)]
            for ti in range(2):
                ps_z = psA.tile([128, CBT], F32, tag="pm", name=f"ps_z{ti}")
                nc.tensor.matmul(ps_z[:, 0:cbt],
                                 w_z_sb[:, ti * 128:(ti + 1) * 128],
                                 hp[:, 0:cb, 3:TP], start=True, stop=True)
                nc.scalar.activation(
                    sz[ti][:, 0:b4, :, :],
                    _ap(ps_z[:], [[2 * T, b4], [1, T], [T, 2]]),
                    AF.Silu)

            # conv via shifted-window matmuls (weights pre-folded w/
            # in_proj); xc2 written in 4-chain layout [p, b4, t, bpair, a]
            xc2 = [sp.tile([128, B4, T, 2, 2], BF16, tag=f"xc{ti}",
                           name=f"xc{ti}_{ch}") for ti in range(2)]
            for ti in range(2):
                for br in range(2):
                    src = hp if br == 0 else hr
                    ps_c = psCv.tile([128, CB, T], F32, tag="pc")
                    for j, k in enumerate((3, 2, 1, 0)):
                        nc.tensor.matmul(ps_c[:, 0:cb, :],
                                         wconv_sb[:, br, ti, k, :],
                                         src[:, 0:cb, k:k + T],
                                         start=(j == 0), stop=(j == 3))
                    nc.scalar.activation(
                        xc2[ti][:, 0:b4, :, :, br],
                        _ap(ps_c[:], [[2 * T, b4], [1, T], [T, 2]]),
                        AF.Silu, bias=convb_sb[:, br, ti, 0:1])

            # xproj -> x_dbl [40, CBT] per branch; B/C staged branch-
            # interleaved [n, b, t, a] via cheap strided ACT copies so the
            # DRAM round-trip DMAs stay fully contiguous.
            dtraw = [None, None]
            bc2 = stp.tile([32, B4, T, 2, 2], BF16, tag="bc2",
                           name=f"bc2_{ch}")
            for br in range(2):
                ps_xd = psA.tile([128, CBT], F32, tag="pm", name=f"ps_xd{br}")
                for ti in range(2):
                    nc.tensor.matmul(ps_xd[0:40, 0:cbt], xw_sb[:, br, ti, :],
                                     _ap(xc2[ti][:],
                                         [[4 * T, b4], [4, T], [2, 2]],
                                         offset=br),
                                     start=(ti == 0), stop=(ti == 1))
                nc.scalar.copy(bc2[:, 0:b4, :, :, br],
                               ps_xd[0:32, 0:cbt].rearrange(
                                   "p (b t x) -> p b t x", t=T, x=2))
                dtraw[br] = stp.tile([RK, CBT], BF16, tag=f"dtraw{br}",
                                     name=f"dtraw{br}_{ch}")
                nc.scalar.copy(dtraw[br][:, 0:cbt], ps_xd[32:40, 0:cbt])

            # B/C broadcast staging (DRAM round-trip); brep+crep loads here
            # (both bufs=2). b1d write + brep broadcast are issued BEFORE
            # the c1d transpose (whose 8-byte-granule descriptors would
            # otherwise delay them in the DMA queues). c1d is written in
            # the segdot layout [half, b4, t, n8, bpair, branch].
            b1d = drp.tile([DS, B4, T, 2, 2], BF16, tag="b1d")
            c1d = drp.tile([2, B4, T, 8, 2, 2], BF16, tag="c1d")
            nc.sync.dma_start(b1d[:, 0:b4], bc2[0:DS, 0:b4])
            NB = B4 * T * 4
            brep = brp.tile([128, DS * CBT * 2], BF16, tag="brep")
            b1f = b1d[:].rearrange("n b t x a -> (n b t x a)")
            nc.sync.dma_start(
                _ap(brep[:], [[NB, DS], [1, b4 * T * 4]]),
                _pbcast(bass.AP(tensor=b1f.tensor, offset=b1f.offset,
                                ap=[[NB, DS], [1, b4 * T * 4]])))
            for hf_ in range(2):
                dst = bass.AP(
                    tensor=c1d[:].tensor,
                    offset=c1d[:].offset + hf_ * (B4 * T * 32),
                    ap=[[4, 8], [32, b4 * T], [1, 4]])
                nc.sync.dma_start(dst, bc2[DS + 8 * hf_:DS + 8 * hf_ + 8,
                                           0:b4])

            # dtproj; dt = ln(1 + exp(x + bias)); dt2/du2 in the 4-chain
            # layout [p, b4, t, bpair, a]
            dt2 = [sp.tile([128, B4, T, 2, 2], BF16, tag=f"dt{ti}",
                           name=f"dt{ti}_{ch}", bufs=1 if ti == 0 else 2)
                   for ti in range(2)]
            for br in range(2):
                for ti in range(2):
                    ps_dt = psA.tile([128, CBT], F32, tag="pm",
                                     name=f"ps_dt{br}{ti}")
                    nc.tensor.matmul(ps_dt[:, 0:cbt],
                                     dtw_sb[:, br, ti * 128:(ti + 1) * 128],
                                     dtraw[br][:, 0:cbt],
                                     start=True, stop=True)
                    slab = dt2[ti][:, 0:b4, :, :, br]
                    nc.scalar.activation(
                        slab,
                        ps_dt[:, 0:cbt].rearrange("p (b t x) -> p b t x",
                                                  t=T, x=2),
                        AF.Exp, bias=dtb_sb[:, br, ti, 0:1])
                    nc.scalar.activation(slab, slab, AF.Ln, bias=1.0)

            # du = dt * xc (bf16, 4-chain layout; xc read strided)
            du2 = [sp.tile([128, B4, T, 2, 2], BF16, tag=f"du{ti}",
                           name=f"du{ti}_{ch}") for ti in range(2)]
            for ti in range(2):
                nc.vector.tensor_mul(du2[ti][:, 0:b4], dt2[ti][:, 0:b4],
                                     xc2[ti][:, 0:b4])

            # dA for ti=0 (parity tile); exp over t in [1, T)
            for n in range(DS):
                nc.scalar.activation(dA0[par][:, n, 0:b4, 1:T, :, :],
                                     dt2[0][:, 0:b4, 1:T, :, :],
                                     AF.Exp, scale=float(a_pow[n]))

            state[ch] = dict(u=u, uf=uf, sz=sz, xc2=xc2, du2=du2, dt2=dt2,
                             b1d=b1d, c1d=c1d, brep=brep)

        def back_pre(ch):
            """crep broadcast + dA1 exps: emitted right after back(ch-1)
            (WAR on dA1/crep) and before front(ch+1), so they are queued
            ahead of the next chunk's ACT/DMA work."""
            b4 = CBS[ch] // 2
            st = state[ch]
            # crep in segdot layout [half, b4, t, n8, bpair, branch]; both
            # sides fully contiguous per half.
            HSEG = B4 * T * 32
            useg = b4 * T * 32
            crep = crp.tile([128, 2 * HSEG], BF16, tag="crep")
            c1f = st["c1d"][:].rearrange("h b t n x a -> (h b t n x a)")
            for hf_ in range(2):
                nc.sync.dma_start(
                    _ap(crep[:], [[1, useg]], offset=hf_ * HSEG),
                    _pbcast(bass.AP(tensor=c1f.tensor,
                                    offset=c1f.offset + hf_ * HSEG,
                                    ap=[[1, useg]])))
            st["crep"] = crep

        def back(ch):
            par = ch % 2
            b0, cb = OFF[ch], CBS[ch]
            b4, cbt = cb // 2, cb * T
            NB = B4 * T * 4            # full per-n block (tile layout)
            UB = b4 * T * 4            # used span per n block
            HSEG = B4 * T * 32
            st = state.pop(ch)
            brepf = st["brep"][:]
            crepf = st["crep"][:]

            # dA for ti=1 (single tile; ACT runs during dBu_0/scan_0)
            for n in range(DS):
                nc.scalar.activation(dA1[:, n, 0:b4, 1:T, :, :],
                                     st["dt2"][1][:, 0:b4, 1:T, :, :],
                                     AF.Exp, scale=float(a_pow[n]))

            ps_o = psO.tile([C, CBT], F32, tag="po", name=f"ps_o{ch}")
            HN = DS // 2
            HSZ = HN * NB
            for ti in range(2):
                du4 = _ap(st["du2"][ti][:], [[4 * T, b4], [1, T * 4]])
                dA = dA0[par] if ti == 0 else dA1
                # h in segdot layout [half, b4, t, n8, bpair, branch]
                h = bp.tile([128, 2, B4, T, 8, 2, 2], BF16, tag="h")
                for hf_ in range(2):
                    dBu = bp.tile([128, HN, B4, T, 2, 2], BF16, tag="dBu")
                    # write compacted (n-stride = UB) so the scan's src1 is
                    # a single contiguous free dim (TTSS src1 must be 1D)
                    nc.vector.tensor_mul(
                        _ap(dBu[:], [[UB, HN], [4 * T, b4], [1, T * 4]]),
                        _zstride(du4, 0, HN),
                        _ap(brepf, [[NB, HN], [4 * T, b4], [1, T * 4]],
                            offset=hf_ * HSZ))
                    for n8 in range(HN):
                        _scan4_emit(
                            nc,
                            _ap(h[:], [[32, b4 * T], [1, 4]],
                                offset=hf_ * HSEG + 4 * n8),
                            _ap(dA[:], [[4, b4 * T], [1, 4]],
                                offset=(hf_ * HN + n8) * NB),
                            _ap(dBu[:], [[1, UB]], offset=n8 * UB))
                # segmented dot with C: ys[half, b4, t, bp, br]
                ys = bp.tile([128, 2, B4, T, 2, 2], BF16, tag="ys")
                for hf_ in range(2):
                    _segdot_emit(
                        nc,
                        _ap(ys[:], [[4, b4 * T], [1, 4]],
                            offset=hf_ * (B4 * T * 4)),
                        _ap(h[:], [[32, b4 * T], [1, 32]],
                            offset=hf_ * HSEG),
                        _ap(crepf, [[32, b4 * T], [1, 32]],
                            offset=hf_ * HSEG))
                yv = _ap(ys[:], [[1, b4 * T * 4]])
                nc.vector.tensor_add(
                    yv, yv, _ap(ys[:], [[1, b4 * T * 4]], offset=B4 * T * 4))
                # stage ypre/yb in the (now dead) ys half-1 slab
                NB2 = B4 * T * 4
                ypre = ys[:, 1, 0:b4, :, :, 0]
                yb = ys[:, 1, 0:b4, :, :, 1]
                nc.vector.scalar_tensor_tensor(
                    ypre,
                    _ap(st["xc2"][ti][:], [[4 * T, b4], [4, T], [2, 2]]),
                    dpc_sb[:, 0, ti, 0:1],
                    ys[:, 0, 0:b4, :, :, 0], ALU.mult, ALU.add)
                nc.vector.scalar_tensor_tensor(
                    yb,
                    _ap(st["xc2"][ti][:], [[4 * T, b4], [4, T], [2, 2]],
                        offset=1),
                    dpc_sb[:, 1, ti, 0:1],
                    ys[:, 0, 0:b4, :, :, 1], ALU.mult, ALU.add)
                # ypre += reverse_t(yb); then gate by silu(z)
                yb_rev = _ap(ys[:], [[4 * T, b4], [-4, T], [2, 2]],
                             offset=NB2 + 1 + 4 * (T - 1))
                nc.vector.tensor_add(ypre, ypre, yb_rev)
                # gate into a separate small tile so the out-proj matmul
                # does not pin the ys tile against the next scan (WAR)
                ypt = sp.tile([128, B4, T, 2], BF16, tag="ypt",
                              name=f"ypt{ti}_{ch}")
                yp_m = _ap(ys[:], [[2, b4 * T * 2]], offset=NB2)
                sz_i = _ap(st["sz"][ti][:], [[1, b4 * T * 2]])
                nc.vector.tensor_mul(
                    _ap(ypt[:], [[1, b4 * T * 2]]), yp_m, sz_i)
                # rhs iterated (b4, bpair, t) so ps_o columns are standard
                # (b, t) token order
                rhs = _ap(ypt[:], [[2 * T, b4], [1, 2], [2, T]])
                nc.tensor.matmul(ps_o[:, 0:cbt], wout_sb[:, ti, :], rhs,
                                 start=(ti == 0), stop=(ti == 1))

            tails[ch] = dict(ps_o=ps_o, uf=st["uf"])

        def back_tail(ch):
            b0, cb = OFF[ch], CBS[ch]
            cbt = cb * T
            tl = tails.pop(ch)
            o_sb = sp.tile([C, CBT], F32, tag="o_sb", name=f"o_sb{ch}")
            nc.scalar.copy(o_sb[:, 0:cbt], tl["ps_o"][:, 0:cbt])
            layernorm(o_sb[:, 0:cbt], ln2g_sb, ln2b_sb, o_sb[:, 0:cbt],
                      trivial=ln_trivial[1], cbt=cbt)
            nc.vector.tensor_add(o_sb[:, 0:cbt], o_sb[:, 0:cbt], tl["uf"])
            nc.sync.dma_start(out[:, b0:b0 + cb, :],
                              o_sb[:, 0:cbt].rearrange("p (b t) -> p b t",
                                                       t=T))

        for ch in range(NCHUNK):
            front(ch)
            if ch > 0:
                back(ch - 1)
            back_pre(ch)
            if ch > 1:
                back_tail(ch - 2)
        back(NCHUNK - 1)
        back_tail(NCHUNK - 2)
        back_tail(NCHUNK - 1)

    nc.finalize()
    return nc


def _prep(inputs):
    f = lambda k: np.ascontiguousarray(np.asarray(inputs[k], np.float32))
    bf = lambda a: np.ascontiguousarray(np.asarray(a, ml_dtypes.bfloat16))
    x = f("x")
    u_all = x.transpose(0, 2, 1, 3).reshape(B * N, T, C)
    u_pad = np.zeros((BSEQ, T, C), np.float32)
    u_pad[:B * N] = u_all
    xin = [bf(u_pad[i * BC:(i + 1) * BC].transpose(2, 0, 1))
           for i in range(NCORES)]

    A = -np.exp(f("A_log"))
    Ab = -np.exp(f("A_b_log"))
    assert np.allclose(A, A[0:1], rtol=1e-5), "A must be d-independent"
    assert np.allclose(Ab, A, rtol=1e-5), "A_b must equal A"
    a_pow = [float(v) for v in A[0]]

    w_in_t = f("in_proj_w").T                      # [C, 2*DI]
    w_in_x = w_in_t[:, :DI]                        # [C, DI]
    cw = np.stack([f("conv_w")[:, 0, :], f("conv_w_b")[:, 0, :]])  # [2,DI,4]
    # wconv[c, br, ti, k, d] = w_in_x[c, ti*128+d] * cw[br, ti*128+d, k]
    wconv = np.einsum('cd,bdk->bkcd', w_in_x, cw)  # [2, 4, C, DI]
    wconv = wconv.reshape(2, 4, C, 2, 128).transpose(2, 0, 3, 1, 4)
    cb = np.stack([f("conv_b"), f("conv_b_b")])[..., None]         # [2,DI,1]
    xw_ro = np.concatenate([f("xproj_w")[RK:], f("xproj_w")[:RK]])
    xw_ro_b = np.concatenate([f("xproj_w_b")[RK:], f("xproj_w_b")[:RK]])
    xwm = np.stack([xw_ro, xw_ro_b]).transpose(0, 2, 1)
    dtwm = np.stack([f("dtproj_w"), f("dtproj_w_b")]).transpose(0, 2, 1)
    dtbm = np.stack([f("dtproj_b"), f("dtproj_b_b")])[..., None]
    shared = {
        "w_z": bf(w_in_t[:, DI:]),
        "wconv": bf(wconv),
        "convb": np.ascontiguousarray(
            cb.reshape(2, 2, 128, 1).transpose(2, 0, 1, 3)),
        "xw": bf(xwm.reshape(2, 2, 128, 40).transpose(2, 0, 1, 3)),
        "dtw": bf(dtwm.transpose(1, 0, 2)),                        # [8,2,256]
        "dtb": np.ascontiguousarray(
            dtbm.reshape(2, 2, 128, 1).transpose(2, 0, 1, 3)),
        "dpc": np.ascontiguousarray(
            np.stack([f("Dp"), f("Dp_b")])[..., None]
            .reshape(2, 2, 128, 1).transpose(2, 0, 1, 3)),
        "wout": bf(
            f("out_proj_w").T.reshape(2, 128, 128).transpose(1, 0, 2)),
        "ln1g": f("ln1_g").reshape(C, 1),
        "ln1b": f("ln1_b").reshape(C, 1),
        "ln2g": f("ln2_g").reshape(C, 1),
        "ln2b": f("ln2_b").reshape(C, 1),
    }
    return xin, shared, a_pow


def _unshard(core_outs):
    y = np.stack(core_outs)                       # [8, C, BC, T]
    y = y.transpose(0, 2, 3, 1).reshape(BSEQ, T, C)[:B * N]
    return np.ascontiguousarray(
        y.reshape(B, N, T, C).transpose(0, 2, 1, 3))


_CACHE = {}


def kernel(_trace=False, **inputs):
    xin, shared, a_pow = _prep(inputs)
    if "prog" not in _CACHE:
        lt = (bool(np.all(inputs["ln1_g"] == 1) and np.all(inputs["ln1_b"] == 0)),
              bool(np.all(inputs["ln2_g"] == 1) and np.all(inputs["ln2_b"] == 0)))
        _CACHE["prog"] = build_program(a_pow, ln_trivial=lt)
    nc = _CACHE["prog"]
    in_maps = [dict(shared, xin=xin[i]) for i in range(NCORES)]
    res = run_bass_kernel_spmd(nc, in_maps, core_ids=list(range(NCORES)),
                               trace=_trace)
    out = _unshard([r["out"] for r in res.results])
    if _trace:
        kernel.last_results = res
    return out



# revision 19
# speedup vs baseline: 1.0122x; 1.0122x over previous
"""BiMamba (bimamba_type='v2') Trainium2 Bass kernel.

Data-parallel over the fused B*N=828 (padded to 896) sequence axis across 8
NeuronCores (112 sequences/core, 8 chunks of 14). Key design points:
  - SCAN4_ANT: custom DVE op (hand-built uOp tables, registered at runtime
    into the ant custom-op rows) runs the selective scan as four
    interleaved recurrences with states in the block-1/3 a/b result flops;
    the 2x_2p perf slot processes packed bf16 pairs at 2 elem/cycle —
    ~3.9x the stock tensor_tensor_scan (which pays a feedback bubble).
    Scan tensors live in a 4-chain layout [p, n, b4, t, bpair, branch]
    (chain = seq-pair half x branch), produced interleaved at the source.
  - depthwise causal conv folded into PE: per tap k, matmul of
    w_in_x[c,d]*conv_w[d,k] against shifted windows of the zero-padded LN1
    output (bwd branch via a reversed padded copy); front-end in bf16.
  - act-table patch: Exp/Ln resolve to natural_log_exp_and_others, killing
    the per-switch ACT_TABLE_LOAD ping-pong; PSUM->SBUF copies on ACT.
  - explicit front/back software pipelining (front(ch+1) emitted before
    back(ch)) with parity-buffered dA/brep tiles.
  - dt = ln(1+exp(.)) (no softplus table); LN rstd = exp(-0.5*ln(var+eps)).
"""

import numpy as np
import ml_dtypes

import concourse.bass as bass
import concourse.tile as tile
from concourse import bacc, mybir
from concourse.bass_utils import run_bass_kernel_spmd

# --- SCAN4_ANT: custom DVE op — 4-interleaved-chain multiply-add scan.
# Stream elements rotate over four independent recurrences (chain = k mod 4):
#   s[c] = d0[k]*s[c] + d1[k]; out[k] = s[c]
# States live in blocks 1/3's a/b result flops. The 1x slot issues 1
# elem/cycle (state re-read 4 cycles after write); the 2x_2p slot processes
# packed bf16 pairs at 2 elems/cycle, pairs alternating between chain groups
# (0,1) and (2,3) so each group's state is re-read 2 cycles after writing.
from dataclasses import dataclass as _dataclass

from concourse import dve_ops as _ops_mod
from concourse.dve_ops import _COMPILE_CACHE as _DVE_CACHE
from concourse.dve_spec import Spec as _Spec, Src0 as _Src0, Src1 as _Src1
from concourse.dve_uop import (
    ENABLE as _EN,
    AluInp as _AluInp,
    AluOp as _AluOp,
    DelayInp as _DelayInp,
    DveOpSpec as _DveOpSpec,
    InpSel as _InpSel,
    OutPath as _OutPath,
    OutSel as _OutSel,
    Trigger as _Trigger,
    UopConfig as _UopConfig,
)

_SCAN_NAME = "SCAN4_ANT"
_SCAN_ROW = 17  # rows 1..16 used by stock OPS; byte-36 row field < 0x20
_SEGDOT_NAME = "SEGDOT8_ANT"
_SEGDOT_ROW = 18


def _uop_1x(chain, init, nxt):
    u = _UopConfig()
    u.enable_input(_InpSel.SRC_0, 0)
    u.enable_input(_InpSel.SRC_1, 1)
    if init:
        u.enable_input(_InpSel.ZERO, 2)
    u.require_inp0 = _EN
    u.require_inp1 = _EN
    u.repeat_count = 1
    u.trigger = (_Trigger.SRC_TENSOR_DONE, _Trigger.COUNT, _Trigger.NONE)
    u.next_uop = (0, nxt, 0)
    u.enable_output(_OutSel.ALU_OUT, _OutPath.WR0_LO)
    mb, ab = (0, 1) if chain < 2 else (2, 3)
    flop_a = chain % 2 == 0
    state_src = _AluInp.PREV_DELAY_1 if init else (
        _AluInp.NEXT_ALU_OUT_A if flop_a else _AluInp.NEXT_ALU_OUT_B)
    for k in range(0, mb):
        u.datapath_config[k].pass_through_alu()
        u.datapath_config[k].pass_through_delay(0)
        if init:
            u.datapath_config[k].pass_through_delay(1)
    u.datapath_config[mb].enable_alu(_AluOp.MULTIPLY, _AluInp.PREV_ALU_OUT,
                                     state_src)
    u.datapath_config[mb].pass_through_delay(0)
    u.datapath_config[ab].enable_alu(_AluOp.ADD, _AluInp.PREV_ALU_OUT,
                                     _AluInp.PREV_DELAY_0)
    if flop_a:
        u.datapath_config[ab].alu_out_a_enable = _EN
    else:
        u.datapath_config[ab].alu_out_b_enable = _EN
    for k in range(ab + 1, 8):
        u.datapath_config[k].pass_through_alu()
    return u


def _uop_2x(group, init, nxt):
    u = _UopConfig()
    u.enable_input(_InpSel.SRC_0, 0)
    u.enable_input(_InpSel.SRC_1, 1)
    u.enable_input(_InpSel.SRC_0_HI, 2)
    u.enable_input(_InpSel.SRC_1_HI, 3)
    if init:
        u.enable_input(_InpSel.ZERO, 4)
    u.require_inp0 = _EN
    u.require_inp1 = _EN
    u.repeat_count = 1
    u.trigger = (_Trigger.SRC_TENSOR_DONE, _Trigger.COUNT, _Trigger.NONE)
    u.next_uop = (0, nxt, 0)
    u.enable_output(_OutSel.DELAY_3, _OutPath.WR0_LO)
    u.enable_output(_OutSel.ALU_OUT, _OutPath.WR0_HI)
    flop_a = group == 0
    st = _AluInp.PREV_DELAY_3 if init else (
        _AluInp.NEXT_ALU_OUT_A if flop_a else _AluInp.NEXT_ALU_OUT_B)
    b0 = u.datapath_config[0]
    b0.enable_alu(_AluOp.MULTIPLY, _AluInp.PREV_ALU_OUT, st)
    b0.pass_through_delay(0, 1, 2)
    if init:
        b0.pass_through_delay(3)
    b1 = u.datapath_config[1]
    b1.enable_alu(_AluOp.ADD, _AluInp.PREV_ALU_OUT, _AluInp.PREV_DELAY_0)
    if flop_a:
        b1.alu_out_a_enable = _EN
    else:
        b1.alu_out_b_enable = _EN
    b1.pass_through_delay(1, 2)
    if init:
        b1.pass_through_delay(3)
    b2 = u.datapath_config[2]
    b2.enable_alu(_AluOp.MULTIPLY, _AluInp.PREV_DELAY_1, st)
    b2.pass_through_delay(2)
    b2.enable_delay_from_src(_DelayInp.PREV_ALU_OUT, 3)
    b3 = u.datapath_config[3]
    b3.enable_alu(_AluOp.ADD, _AluInp.PREV_ALU_OUT, _AluInp.PREV_DELAY_2)
    if flop_a:
        b3.alu_out_a_enable = _EN
    else:
        b3.alu_out_b_enable = _EN
    b3.pass_through_delay(3)
    for k in range(4, 8):
        u.datapath_config[k].pass_through_alu()
        u.datapath_config[k].pass_through_delay(3)
    return u


# --- SEGDOT8_ANT: segmented dot-product. Stream = segments of 32 elements
# ([n=8 outer] x [quad=4 inner]); the 4 quad items are independent
# accumulators (chains); output = 4 values per segment:
#   out[seg, q] = sum_n in0[seg, n, q] * in1[seg, n, q]
# In 2x mode each cycle processes a packed quad-pair; groups (q0,q1)/(q2,q3)
# alternate. acc_lo lives in block-2 a/b flops (read by block-1's ADD via
# NEXT_ALU_OUT), acc_hi in block-5 a/b flops.


def _sd_uop_2x(group, init, emit, nxt):
    u = _UopConfig()
    u.enable_input(_InpSel.SRC_0, 0)
    u.enable_input(_InpSel.SRC_1, 1)
    u.enable_input(_InpSel.SRC_0_HI, 2)
    u.enable_input(_InpSel.SRC_1_HI, 3)
    if init:
        u.enable_input(_InpSel.ZERO, 4)
    u.require_inp0 = _EN
    u.require_inp1 = _EN
    u.repeat_count = 1
    u.trigger = (_Trigger.SRC_TENSOR_DONE, _Trigger.COUNT, _Trigger.NONE)
    u.next_uop = (0, nxt, 0)
    if emit:
        u.enable_output(_OutSel.DELAY_3, _OutPath.WR0_LO)
        u.enable_output(_OutSel.ALU_OUT, _OutPath.WR0_HI)
    flop_a = group == 0
    st = _AluInp.PREV_DELAY_3 if init else _AluInp.NEXT_ALU_OUT_A
    if not init and not flop_a:
        st = _AluInp.NEXT_ALU_OUT_B
    # b0: m_lo = h_lo * c_lo   (delay0 = c_lo consumed here)
    b0 = u.datapath_config[0]
    b0.enable_alu(_AluOp.MULTIPLY, _AluInp.PREV_ALU_OUT, _AluInp.PREV_DELAY_0)
    b0.pass_through_delay(1, 2)
    if init:
        b0.pass_through_delay(3)
    # b1: acc_lo' = m_lo + acc_lo (b2's a/b flop; ZERO via delay3 on init)
    b1 = u.datapath_config[1]
    b1.enable_alu(_AluOp.ADD, _AluInp.PREV_ALU_OUT, st)
    b1.pass_through_delay(1, 2)
    if init:
        b1.pass_through_delay(3)
    # b2: bypass acc_lo' into b2's a/b flop
    b2 = u.datapath_config[2]
    b2.pass_through_alu()
    if flop_a:
        b2.alu_out_a_enable = _EN
    else:
        b2.alu_out_b_enable = _EN
    b2.pass_through_delay(1, 2)
    if init:
        b2.pass_through_delay(3)
    # b3: m_hi = h_hi * c_hi; on emit also stage acc_lo' into delay3
    b3 = u.datapath_config[3]
    b3.enable_alu(_AluOp.MULTIPLY, _AluInp.PREV_DELAY_1, _AluInp.PREV_DELAY_2)
    if emit:
        b3.enable_delay_from_src(_DelayInp.PREV_ALU_OUT, 3)
    elif init:
        b3.pass_through_delay(3)
    # b4: acc_hi' = m_hi + acc_hi (b5's a/b flop)
    b4 = u.datapath_config[4]
    st_hi = _AluInp.PREV_DELAY_3 if init else _AluInp.NEXT_ALU_OUT_A
    if not init and not flop_a:
        st_hi = _AluInp.NEXT_ALU_OUT_B
    b4.enable_alu(_AluOp.ADD, _AluInp.PREV_ALU_OUT, st_hi)
    if emit:
        b4.pass_through_delay(3)
    # b5: bypass acc_hi' into b5's a/b flop
    b5 = u.datapath_config[5]
    b5.pass_through_alu()
    if flop_a:
        b5.alu_out_a_enable = _EN
    else:
        b5.alu_out_b_enable = _EN
    if emit:
        b5.pass_through_delay(3)
    for k in range(6, 8):
        u.datapath_config[k].pass_through_alu()
        if emit:
            u.datapath_config[k].pass_through_delay(3)
    return u


def _sd_uop_1x(chain, init, emit, nxt):
    u = _UopConfig()
    u.enable_input(_InpSel.SRC_0, 0)
    u.enable_input(_InpSel.SRC_1, 1)
    if init:
        u.enable_input(_InpSel.ZERO, 2)
    u.require_inp0 = _EN
    u.require_inp1 = _EN
    u.repeat_count = 1
    u.trigger = (_Trigger.SRC_TENSOR_DONE, _Trigger.COUNT, _Trigger.NONE)
    u.next_uop = (0, nxt, 0)
    if emit:
        u.enable_output(_OutSel.ALU_OUT, _OutPath.WR0_LO)
    flop_a = chain % 2 == 0
    ab = 1 if chain < 2 else 3         # ADD block; store block = ab+1
    st = _AluInp.PREV_DELAY_1 if init else (
        _AluInp.NEXT_ALU_OUT_A if flop_a else _AluInp.NEXT_ALU_OUT_B)
    b0 = u.datapath_config[0]
    b0.enable_alu(_AluOp.MULTIPLY, _AluInp.PREV_ALU_OUT, _AluInp.PREV_DELAY_0)
    if init:
        b0.pass_through_delay(1)
    for k in range(1, ab):
        u.datapath_config[k].pass_through_alu()
        if init:
            u.datapath_config[k].pass_through_delay(1)
    u.datapath_config[ab].enable_alu(_AluOp.ADD, _AluInp.PREV_ALU_OUT, st)
    bs = u.datapath_config[ab + 1]
    bs.pass_through_alu()
    if flop_a:
        bs.alu_out_a_enable = _EN
    else:
        bs.alu_out_b_enable = _EN
    for k in range(ab + 2, 8):
        u.datapath_config[k].pass_through_alu()
    return u


@_dataclass(frozen=True)
class _ShimSpec:
    accum: object = None


class _ScanOp:
    name = _SCAN_NAME
    subdim = False
    spec = _ShimSpec()
    perf_en: dict = {}

    def compile(self, ver):
        key = (self.name, ver)
        if key not in _DVE_CACHE:
            uops = [
                _uop_1x(0, True, 1), _uop_1x(1, True, 2),
                _uop_1x(2, True, 3), _uop_1x(3, True, 4),
                _uop_1x(0, False, 5), _uop_1x(1, False, 6),
                _uop_1x(2, False, 7), _uop_1x(3, False, 4),
            ]
            u2 = [
                _uop_2x(0, True, 1), _uop_2x(1, True, 2),
                _uop_2x(0, False, 3), _uop_2x(1, False, 2),
                _uop_2x(0, False, 3), _uop_2x(1, False, 2),
                _uop_2x(0, False, 3), _uop_2x(1, False, 2),
            ]
            u2p = [
                _uop_2x(0, True, 1), _uop_2x(1, True, 2),
                _uop_2x(0, False, 3), _uop_2x(1, False, 2),
                _uop_2x(0, False, 3), _uop_2x(1, False, 2),
                _uop_2x(0, False, 3), _uop_2x(1, False, 2),
            ]
            _DVE_CACHE[key] = _DveOpSpec(
                name=self.name, opcode=_SCAN_ROW, uops=uops,
                uops_2x=u2, uops_2x_2p=u2p, perf_max=2, rd1_en=True)
        return _DVE_CACHE[key]


_SCAN4 = _ScanOp()


class _SegDotOp:
    name = _SEGDOT_NAME
    subdim = False
    spec = _ShimSpec()
    perf_en: dict = {}

    def compile(self, ver):
        key = (self.name, ver)
        if key not in _DVE_CACHE:
            def chain2x():
                us = [_sd_uop_2x(0, True, False, 2)]     # entry
                us.append(_sd_uop_2x(0, True, False, 2))  # loop initA
                us.append(_sd_uop_2x(1, True, False, 3))  # initB
                for j in range(6):
                    us.append(_sd_uop_2x(0, False, False, 4 + 2 * j))
                    us.append(_sd_uop_2x(1, False, False, 5 + 2 * j))
                us.append(_sd_uop_2x(0, False, True, 16))
                us.append(_sd_uop_2x(1, False, True, 1))
                return us

            # The op's APs statically satisfy the 2X_1PORT trigger
            # conditions (2B dtype, unit inner step, 4B alignment), so the
            # REGULAR slot is never reached — fill it with the 2x chain to
            # stay inside the 256-entry control table. uops_2x_2p=None
            # reuses the 2X_1PORT continuation slots.
            _DVE_CACHE[key] = _DveOpSpec(
                name=self.name, opcode=_SEGDOT_ROW, uops=chain2x(),
                uops_2x=chain2x(), uops_2x_2p=None, perf_max=2,
                rd1_en=True)
        return _DVE_CACHE[key]


_SEGDOT = _SegDotOp()


def _segdot_register():
    if _SEGDOT_NAME in _ops_mod._SUB_OPCODE_FOR_NAME:
        return
    _ops_mod._SUB_OPCODE_FOR_NAME[_SEGDOT_NAME] = _SEGDOT_ROW
    _ops_mod.OPS.append(_SEGDOT)
    _ops_mod.CUSTOM_DVE_SPECS[_SEGDOT_NAME] = _Spec(
        body=_Src0 * _Src1,
        reference=lambda in0, in1, s0, s1, imm2: in0 * in1,
    )


def _segdot_emit(nc, out, in0, in1):
    _segdot_register()
    from concourse import bass_isa
    from concourse.bass_utils import dve_ver_for

    v = nc.vector
    if _SEGDOT.name not in v.bass.m.ant_custom_dve_ops:
        v.bass.m.ant_custom_dve_ops = sorted(
            {*v.bass.m.ant_custom_dve_ops, _SEGDOT.name})
    _SEGDOT.compile(dve_ver_for(v.bass.trn_type))
    shape = bass_isa.CustomDveShape.TTSS
    isa_opcode = v.bass.isa.Opcode[
        f"NEURON_ISA_TPB_OPCODE_CUSTOM_DVE_ANT_{shape.slot()}"].value
    imm = mybir.ImmediateValue(dtype=mybir.dt.float32, value=0.0)
    inst = bass_isa.InstCustomDveAnt(
        name=v.bass.get_next_instruction_name(),
        op_name=_SEGDOT.name,
        rd1_en=True,
        subdim=0,
        imm2=0.0,
        shape=shape,
        row=_SEGDOT_ROW,
        isa_opcode=isa_opcode,
        ins=[v.lower_ap(in0, for_isa=True),
             v.lower_ap(in1, for_isa=True), imm, imm],
        outs=[v.lower_ap(out, for_isa=True)],
    )
    inst.perf_max = 2
    return v.add_instruction(inst)


def _scan4_register():
    if _SCAN_NAME in _ops_mod._SUB_OPCODE_FOR_NAME:
        return
    _ops_mod._SUB_OPCODE_FOR_NAME[_SCAN_NAME] = _SCAN_ROW
    _ops_mod.OPS.append(_SCAN4)
    _ops_mod.CUSTOM_DVE_SPECS[_SCAN_NAME] = _Spec(
        body=_Src0 * _Src1,
        reference=lambda in0, in1, s0, s1, imm2: in0 * in1,
    )


def _scan4_emit(nc, out, d0, d1):
    _scan4_register()
    from concourse import bass_isa
    from concourse.bass_utils import dve_ver_for

    v = nc.vector
    if _SCAN4.name not in v.bass.m.ant_custom_dve_ops:
        v.bass.m.ant_custom_dve_ops = sorted(
            {*v.bass.m.ant_custom_dve_ops, _SCAN4.name})
    _SCAN4.compile(dve_ver_for(v.bass.trn_type))
    shape = bass_isa.CustomDveShape.TTSS
    isa_opcode = v.bass.isa.Opcode[
        f"NEURON_ISA_TPB_OPCODE_CUSTOM_DVE_ANT_{shape.slot()}"].value
    imm = mybir.ImmediateValue(dtype=mybir.dt.float32, value=0.0)
    inst = bass_isa.InstCustomDveAnt(
        name=v.bass.get_next_instruction_name(),
        op_name=_SCAN4.name,
        rd1_en=True,
        subdim=0,
        imm2=0.0,
        shape=shape,
        row=_SCAN_ROW,
        isa_opcode=isa_opcode,
        ins=[v.lower_ap(d0, for_isa=True),
             v.lower_ap(d1, for_isa=True), imm, imm],
        outs=[v.lower_ap(out, for_isa=True)],
    )
    inst.perf_max = 2
    return v.add_instruction(inst)



F32 = mybir.dt.float32
BF16 = mybir.dt.bfloat16
AF = mybir.ActivationFunctionType
ALU = mybir.AluOpType

B, T, N, C = 4, 24, 207, 128
DI = 256
DS = 16
RK = 8
EPS = 1e-5
NCORES = 8
BSEQ = 832
BC = BSEQ // NCORES          # 104 sequences per core (828 real + 4 pad)
NCHUNK = 8
CBS = (14, 14, 14, 14, 14, 14, 10, 10)   # per-chunk seqs (must be even)
OFF = tuple(sum(CBS[:i]) for i in range(NCHUNK))
CB = max(CBS)                # tile-shape maximum
B4 = CB // 2                 # sequence pairs (scan chain interleave)
CBT = CB * T                 # tokens per max chunk
TP = T + 3                   # left-padded time for causal conv windows

# --- act-table patch: make the set chooser pick natural_log_exp_and_others
# for both Exp and Ln (otherwise it alternates exp_and_others/natural_log
# and reloads tables on every switch).
import concourse.bacc as _bacc_mod
from concourse.hw_specs import get_activation_tables as _orig_gat


def _patched_gat(arch):
    t = dict(_orig_gat(arch))
    for nm, drop in (("exp_and_others", AF.Exp), ("exp_and_friends", AF.Exp),
                     ("natural_log", AF.Ln)):
        if nm in t:
            t[nm] = set(t[nm]) - {drop}
    return t


_bacc_mod.get_activation_tables = _patched_gat


def _pbcast(ap, parts=128):
    a = [[0, parts]] + [list(x) for x in ap.ap]
    return bass.AP(tensor=ap.tensor, offset=ap.offset, ap=a)


def _rev_t(ap):
    a = [list(x) for x in ap.ap]
    st, ct = a[-1]
    off = ap.offset + st * (ct - 1)
    a[-1] = [-st, ct]
    return bass.AP(tensor=ap.tensor, offset=off, ap=a)


def _zstride(ap, dim, count):
    a = [list(x) for x in ap.ap]
    a.insert(1 + dim, [0, count])
    return bass.AP(tensor=ap.tensor, offset=ap.offset, ap=a)


def _ap(base, dims, offset=0):
    """AP over base's tensor: keep base's partition dim, explicit free dims
    [[stride, count], ...], extra element offset."""
    return bass.AP(tensor=base.tensor, offset=base.offset + offset,
                   ap=[list(base.ap[0])] + [list(d) for d in dims])


def build_program(a_pow, ln_trivial=(False, False)):
    nc = bacc.Bacc("TRN2", target_bir_lowering=False, debug=False,
                   enable_asserts=False, num_devices=NCORES)

    def din(name, shape, dt=F32):
        return nc.dram_tensor(name, shape, dt, kind="ExternalInput").ap()

    xin = din("xin", [C, BC, T], BF16)
    w_z = din("w_z", [C, 2 * C], BF16)            # z half of in_proj
    wconv = din("wconv", [C, 2, 2, 4, C], BF16)   # [c, br, ti, k, d]
    convb = din("convb", [128, 2, 2, 1])
    xw = din("xw", [128, 2, 2, 40], BF16)
    dtw = din("dtw", [RK, 2, DI], BF16)
    dtb = din("dtb", [128, 2, 2, 1])
    dpc = din("dpc", [128, 2, 2, 1])
    wout = din("wout", [128, 2, C], BF16)
    ln1g = din("ln1g", [C, 1])
    ln1b = din("ln1b", [C, 1])
    ln2g = din("ln2g", [C, 1])
    ln2b = din("ln2b", [C, 1])
    out = nc.dram_tensor("out", [C, BC, T], F32, kind="ExternalOutput").ap()

    with tile.TileContext(nc) as tc, \
         tc.tile_pool(name="weights", bufs=1) as wp, \
         tc.tile_pool(name="small", bufs=2) as sp, \
         tc.tile_pool(name="stats", bufs=2) as stp, \
         tc.tile_pool(name="dbu", bufs=1) as bp, \
         tc.tile_pool(name="brep", bufs=2) as brp, \
         tc.tile_pool(name="crep", bufs=1) as crp, \
         tc.tile_pool(name="dram", bufs=2, space="DRAM") as drp, \
         tc.tile_pool(name="psA", bufs=2, space="PSUM") as psA, \
         tc.tile_pool(name="psCv", bufs=2, space="PSUM") as psCv, \
         tc.tile_pool(name="psB", bufs=2, space="PSUM") as psB, \
         tc.tile_pool(name="psO", bufs=2, space="PSUM") as psO:

        pre_u = {}
        for ch0 in range(2):
            t = sp.tile([C, CB, T], BF16, tag="u", name=f"u{ch0}", bufs=3)
            nc.sync.dma_start(t[:, 0:CBS[ch0], :],
                              xin[:, OFF[ch0]:OFF[ch0] + CBS[ch0], :])
            pre_u[ch0] = t

        def load_w(name, ap_src, shape, dt=F32):
            t = wp.tile(shape, dt, tag=name, name=name)
            nc.sync.dma_start(t[:], ap_src)
            return t

        w_z_sb = load_w("w_z", w_z, [C, 2 * C], BF16)
        # wconv is 512KB — split the load across 8 DMAs so it spreads over
        # parallel queues instead of serializing ~23us on one
        wconv_sb = wp.tile([C, 2, 2, 4, C], BF16, tag="wconv", name="wconv")
        for _br in range(2):
            for _ti in range(2):
                for _kh in range(2):
                    nc.sync.dma_start(
                        wconv_sb[:, _br, _ti, 2 * _kh:2 * _kh + 2, :],
                        wconv[:, _br, _ti, 2 * _kh:2 * _kh + 2, :])
        convb_sb = load_w("convb", convb, [128, 2, 2, 1])
        xw_sb = load_w("xw", xw, [128, 2, 2, 40], BF16)
        dtw_sb = load_w("dtw", dtw, [RK, 2, DI], BF16)
        dtb_sb = load_w("dtb", dtb, [128, 2, 2, 1])
        dpc_sb = load_w("dpc", dpc, [128, 2, 2, 1])
        wout_sb = load_w("wout", wout, [128, 2, C], BF16)
        ones_bf = wp.tile([C, 1], BF16, tag="ones_bf")
        nc.vector.memset(ones_bf[:], 1.0 / C)
        ln1g_sb = load_w("ln1g", ln1g, [C, 1])
        ln1b_sb = load_w("ln1b", ln1b, [C, 1])
        ln2g_sb = load_w("ln2g", ln2g, [C, 1])
        ln2b_sb = load_w("ln2b", ln2b, [C, 1])
        ones_sb = wp.tile([C, 1], F32, tag="ones")
        nc.vector.memset(ones_sb[:], 1.0 / C)
        eps_sb = wp.tile([C, 1], F32, tag="eps")
        nc.vector.memset(eps_sb[:], EPS)
        ones_row = wp.tile([1, C], F32, tag="ones_row")
        nc.vector.memset(ones_row[:], 1.0)

        # persistent padded LN1 outputs (fwd + reversed), 2 parities
        hlnp = [wp.tile([C, CB, TP], BF16, tag=f"hlnp{i}", name=f"hlnp{i}")
                for i in range(2)]
        hlnr = [wp.tile([C, CB, TP], BF16, tag=f"hlnr{i}", name=f"hlnr{i}")
                for i in range(2)]
        for tl in hlnp + hlnr:
            nc.gpsimd.memset(tl[:, :, 0:3], 0.0)

        # persistent dA tiles in 4-chain interleaved layout
        # [p, n, b4, t, bpair, a]: ti=0 double-buffered (exps in front),
        # ti=1 single (exps at back start). t=0 column zero = segment reset.
        dA0 = [wp.tile([128, DS, B4, T, 2, 2], BF16, tag=f"dA0_{i}",
                       name=f"dA0_{i}") for i in range(2)]
        dA1 = wp.tile([128, DS, B4, T, 2, 2], BF16, tag="dA1", name="dA1")
        for tl in dA0 + [dA1]:
            nc.gpsimd.memset(tl[:, :, :, 0:1, :, :], 0.0)

        def layernorm(src_f32, g_sb, b_sb, dst, trivial=False, cbt=CBT):
            """LN over channel (partition) dim of src [C, CBT] -> dst view."""
            sq = sp.tile([C, CBT], BF16, tag="ln_sq", bufs=1)
            nc.scalar.activation(sq[:, 0:cbt], src_f32, AF.Square)
            ps_s = psA.tile([128, CBT], F32, tag="pm", name="ps_s")
            ps_q = psA.tile([128, CBT], F32, tag="pm", name="ps_q")
            ones_like = (ones_bf if src_f32.tensor.dtype == BF16
                         else ones_sb)
            nc.tensor.matmul(ps_s[0:1, 0:cbt], ones_like[:], src_f32,
                             start=True, stop=True)
            nc.tensor.matmul(ps_q[0:1, 0:cbt], ones_bf[:], sq[:, 0:cbt],
                             start=True, stop=True)
            mean = stp.tile([1, CBT], F32, tag="mean")
            nc.scalar.copy(mean[:, 0:cbt], ps_s[0:1, 0:cbt])
            var = stp.tile([1, CBT], F32, tag="var")
            nc.scalar.copy(var[:, 0:cbt], ps_q[0:1, 0:cbt])
            m2 = stp.tile([1, CBT], F32, tag="m2")
            nc.scalar.square(m2[:, 0:cbt], mean[:, 0:cbt])
            nc.vector.tensor_sub(var[:, 0:cbt], var[:, 0:cbt], m2[:, 0:cbt])
            # rstd = (var+eps)^-0.5 = exp(-0.5*ln(var+eps))
            nc.scalar.activation(var[:, 0:cbt], var[:, 0:cbt], AF.Ln,
                                 bias=eps_sb[0:1, 0:1])
            nc.scalar.activation(var[:, 0:cbt], var[:, 0:cbt], AF.Exp,
                                 scale=-0.5)
            mean_r = psB.tile([C, CBT], F32, tag="pb", name="mean_r")
            nc.tensor.matmul(mean_r[:, 0:cbt], ones_row[:], mean[:, 0:cbt],
                             start=True, stop=True)
            rstd_r = psB.tile([C, CBT], F32, tag="pb", name="rstd_r")
            nc.tensor.matmul(rstd_r[:, 0:cbt], ones_row[:], var[:, 0:cbt],
                             start=True, stop=True)
            tmp = sp.tile([C, CBT], BF16, tag="ln_tmp", bufs=1)
            nc.vector.tensor_sub(tmp[:, 0:cbt], src_f32, mean_r[:, 0:cbt])
            if trivial:
                # g==1, b==0: write the normalize directly to dst
                rv = rstd_r[:, 0:cbt]
                if len(dst.shape) == 3:
                    tv = tmp[:, 0:cbt].rearrange("p (b t) -> p b t",
                                                 t=dst.shape[2])
                    rv = rv.rearrange("p (b t) -> p b t", t=dst.shape[2])
                else:
                    tv = tmp[:, 0:cbt]
                nc.vector.tensor_mul(dst, tv, rv)
                return
            nc.vector.tensor_mul(tmp[:, 0:cbt], tmp[:, 0:cbt],
                                 rstd_r[:, 0:cbt])
            tv = tmp[:, 0:cbt]
            if len(dst.shape) == 3:
                tv = tv.rearrange("p (b t) -> p b t", t=dst.shape[2])
            nc.vector.tensor_scalar(dst, tv, g_sb[:, 0:1], b_sb[:, 0:1],
                                    ALU.mult, ALU.add)

        state = {}
        tails = {}

        def front(ch):
            par = ch % 2
            b0, cb = OFF[ch], CBS[ch]
            b4, cbt = cb // 2, cb * T
            if ch in pre_u:
                u = pre_u.pop(ch)
            else:
                u = sp.tile([C, CB, T], BF16, tag="u", name=f"u{ch}",
                            bufs=3)
                nc.sync.dma_start(u[:, 0:cb, :], xin[:, b0:b0 + cb, :])
            uf = u[:, 0:cb, :].rearrange("p b t -> p (b t)")

            hp, hr = hlnp[par], hlnr[par]
            layernorm(uf, ln1g_sb, ln1b_sb, hp[:, 0:cb, 3:TP],
                      trivial=ln_trivial[0], cbt=cbt)
            # reversed copy for the bwd-branch conv windows
            nc.scalar.copy(hr[:, 0:cb, 3:TP], _rev_t(hp[:, 0:cb, 3:TP]))

            # z half + silu gate
            sz = [sp.tile([128, B4, T, 2], BF16, tag=f"sz{ti}",
                          name=f"sz{ti}_{ch}") for ti in range(2)]
            for ti in range(2):
                ps_z = psA.tile([128, CBT], F32, tag="pm", name=f"ps_z{ti}")
                nc.tensor.matmul(ps_z[:, 0:cbt],
                                 w_z_sb[:, ti * 128:(ti + 1) * 128],
                                 hp[:, 0:cb, 3:TP], start=True, stop=True)
                nc.scalar.activation(
                    sz[ti][:, 0:b4, :, :],
                    _ap(ps_z[:], [[2 * T, b4], [1, T], [T, 2]]),
                    AF.Silu)

            # conv via shifted-window matmuls (weights pre-folded w/
            # in_proj); xc2 written in 4-chain layout [p, b4, t, bpair, a]
            xc2 = [sp.tile([128, B4, T, 2, 2], BF16, tag=f"xc{ti}",
                           name=f"xc{ti}_{ch}") for ti in range(2)]
            for ti in range(2):
                for br in range(2):
                    src = hp if br == 0 else hr
                    ps_c = psCv.tile([128, CB, T], F32, tag="pc")
                    for j, k in enumerate((3, 2, 1, 0)):
                        nc.tensor.matmul(ps_c[:, 0:cb, :],
                                         wconv_sb[:, br, ti, k, :],
                                         src[:, 0:cb, k:k + T],
                                         start=(j == 0), stop=(j == 3))
                    nc.scalar.activation(
                        xc2[ti][:, 0:b4, :, :, br],
                        _ap(ps_c[:], [[2 * T, b4], [1, T], [T, 2]]),
                        AF.Silu, bias=convb_sb[:, br, ti, 0:1])

            # xproj -> x_dbl [40, CBT] per branch; B/C staged branch-
            # interleaved [n, b, t, a] via cheap strided ACT copies so the
            # DRAM round-trip DMAs stay fully contiguous.
            dtraw = [None, None]
            bc2 = stp.tile([32, B4, T, 2, 2], BF16, tag="bc2",
                           name=f"bc2_{ch}")
            for br in range(2):
                ps_xd = psA.tile([128, CBT], F32, tag="pm", name=f"ps_xd{br}")
                for ti in range(2):
                    nc.tensor.matmul(ps_xd[0:40, 0:cbt], xw_sb[:, br, ti, :],
                                     _ap(xc2[ti][:],
                                         [[4 * T, b4], [4, T], [2, 2]],
                                         offset=br),
                                     start=(ti == 0), stop=(ti == 1))
                nc.scalar.copy(bc2[:, 0:b4, :, :, br],
                               ps_xd[0:32, 0:cbt].rearrange(
                                   "p (b t x) -> p b t x", t=T, x=2))
                dtraw[br] = stp.tile([RK, CBT], BF16, tag=f"dtraw{br}",
                                     name=f"dtraw{br}_{ch}")
                nc.scalar.copy(dtraw[br][:, 0:cbt], ps_xd[32:40, 0:cbt])

            # B/C broadcast staging (DRAM round-trip); brep+crep loads here
            # (both bufs=2). b1d write + brep broadcast are issued BEFORE
            # the c1d transpose (whose 8-byte-granule descriptors would
            # otherwise delay them in the DMA queues). c1d is written in
            # the segdot layout [half, b4, t, n8, bpair, branch].
            b1d = drp.tile([DS, B4, T, 2, 2], BF16, tag="b1d")
            c1d = drp.tile([2, B4, T, 8, 2, 2], BF16, tag="c1d")
            nc.sync.dma_start(b1d[:, 0:b4], bc2[0:DS, 0:b4])
            NB = B4 * T * 4
            brep = brp.tile([128, DS * CBT * 2], BF16, tag="brep")
            b1f = b1d[:].rearrange("n b t x a -> (n b t x a)")
            nc.sync.dma_start(
                _ap(brep[:], [[NB, DS], [1, b4 * T * 4]]),
                _pbcast(bass.AP(tensor=b1f.tensor, offset=b1f.offset,
                                ap=[[NB, DS], [1, b4 * T * 4]])))
            for hf_ in range(2):
                dst = bass.AP(
                    tensor=c1d[:].tensor,
                    offset=c1d[:].offset + hf_ * (B4 * T * 32),
                    ap=[[4, 8], [32, b4 * T], [1, 4]])
                nc.sync.dma_start(dst, bc2[DS + 8 * hf_:DS + 8 * hf_ + 8,
                                           0:b4])

            # dtproj; dt = ln(1 + exp(x + bias)); dt2/du2 in the 4-chain
            # layout [p, b4, t, bpair, a]
            dt2 = [sp.tile([128, B4, T, 2, 2], BF16, tag=f"dt{ti}",
                           name=f"dt{ti}_{ch}", bufs=1 if ti == 0 else 2)
                   for ti in range(2)]
            for br in range(2):
                for ti in range(2):
                    ps_dt = psA.tile([128, CBT], F32, tag="pm",
                                     name=f"ps_dt{br}{ti}")
                    nc.tensor.matmul(ps_dt[:, 0:cbt],
                                     dtw_sb[:, br, ti * 128:(ti + 1) * 128],
                                     dtraw[br][:, 0:cbt],
                                     start=True, stop=True)
                    slab = dt2[ti][:, 0:b4, :, :, br]
                    nc.scalar.activation(
                        slab,
                        ps_dt[:, 0:cbt].rearrange("p (b t x) -> p b t x",
                                                  t=T, x=2),
                        AF.Exp, bias=dtb_sb[:, br, ti, 0:1])
                    nc.scalar.activation(slab, slab, AF.Ln, bias=1.0)

            # du = dt * xc (bf16, 4-chain layout; xc read strided)
            du2 = [sp.tile([128, B4, T, 2, 2], BF16, tag=f"du{ti}",
                           name=f"du{ti}_{ch}") for ti in range(2)]
            for ti in range(2):
                nc.vector.tensor_mul(du2[ti][:, 0:b4], dt2[ti][:, 0:b4],
                                     xc2[ti][:, 0:b4])

            # dA for ti=0 (parity tile); exp over t in [1, T)
            for n in range(DS):
                nc.scalar.activation(dA0[par][:, n, 0:b4, 1:T, :, :],
                                     dt2[0][:, 0:b4, 1:T, :, :],
                                     AF.Exp, scale=float(a_pow[n]))

            state[ch] = dict(u=u, uf=uf, sz=sz, xc2=xc2, du2=du2, dt2=dt2,
                             b1d=b1d, c1d=c1d, brep=brep)

        def back_pre(ch):
            """crep broadcast + dA1 exps: emitted right after back(ch-1)
            (WAR on dA1/crep) and before front(ch+1), so they are queued
            ahead of the next chunk's ACT/DMA work."""
            b4 = CBS[ch] // 2
            st = state[ch]
            # crep in segdot layout [half, b4, t, n8, bpair, branch]; both
            # sides fully contiguous per half.
            HSEG = B4 * T * 32
            useg = b4 * T * 32
            crep = crp.tile([128, 2 * HSEG], BF16, tag="crep")
            c1f = st["c1d"][:].rearrange("h b t n x a -> (h b t n x a)")
            for hf_ in range(2):
                nc.sync.dma_start(
                    _ap(crep[:], [[1, useg]], offset=hf_ * HSEG),
                    _pbcast(bass.AP(tensor=c1f.tensor,
                                    offset=c1f.offset + hf_ * HSEG,
                                    ap=[[1, useg]])))
            st["crep"] = crep

        def back(ch):
            par = ch % 2
            b0, cb = OFF[ch], CBS[ch]
            b4, cbt = cb // 2, cb * T
            NB = B4 * T * 4            # full per-n block (tile layout)
            UB = b4 * T * 4            # used span per n block
            HSEG = B4 * T * 32
            st = state.pop(ch)
            brepf = st["brep"][:]
            crepf = st["crep"][:]

            # dA for ti=1 (single tile; ACT runs during dBu_0/scan_0)
            for n in range(DS):
                nc.scalar.activation(dA1[:, n, 0:b4, 1:T, :, :],
                                     st["dt2"][1][:, 0:b4, 1:T, :, :],
                                     AF.Exp, scale=float(a_pow[n]))

            ps_o = psO.tile([C, CBT], F32, tag="po", name=f"ps_o{ch}")
            HN = DS // 2
            HSZ = HN * NB
            for ti in range(2):
                du4 = _ap(st["du2"][ti][:], [[4 * T, b4], [1, T * 4]])
                dA = dA0[par] if ti == 0 else dA1
                # h in segdot layout [half, b4, t, n8, bpair, branch]
                h = bp.tile([128, 2, B4, T, 8, 2, 2], BF16, tag="h")
                for hf_ in range(2):
                    dBu = bp.tile([128, HN, B4, T, 2, 2], BF16, tag="dBu")
                    # write compacted (n-stride = UB) so the scan's src1 is
                    # a single contiguous free dim (TTSS src1 must be 1D)
                    nc.vector.tensor_mul(
                        _ap(dBu[:], [[UB, HN], [4 * T, b4], [1, T * 4]]),
                        _zstride(du4, 0, HN),
                        _ap(brepf, [[NB, HN], [4 * T, b4], [1, T * 4]],
                            offset=hf_ * HSZ))
                    for n8 in range(HN):
                        _scan4_emit(
                            nc,
                            _ap(h[:], [[32, b4 * T], [1, 4]],
                                offset=hf_ * HSEG + 4 * n8),
                            _ap(dA[:], [[4, b4 * T], [1, 4]],
                                offset=(hf_ * HN + n8) * NB),
                            _ap(dBu[:], [[1, UB]], offset=n8 * UB))
                # segmented dot with C: ys[half, b4, t, bp, br]
                ys = bp.tile([128, 2, B4, T, 2, 2], BF16, tag="ys")
                for hf_ in range(2):
                    _segdot_emit(
                        nc,
                        _ap(ys[:], [[4, b4 * T], [1, 4]],
                            offset=hf_ * (B4 * T * 4)),
                        _ap(h[:], [[32, b4 * T], [1, 32]],
                            offset=hf_ * HSEG),
                        _ap(crepf, [[32, b4 * T], [1, 32]],
                            offset=hf_ * HSEG))
                yv = _ap(ys[:], [[1, b4 * T * 4]])
                nc.vector.tensor_add(
                    yv, yv, _ap(ys[:], [[1, b4 * T * 4]], offset=B4 * T * 4))
                # stage ypre/yb in the (now dead) ys half-1 slab
                NB2 = B4 * T * 4
                ypre = ys[:, 1, 0:b4, :, :, 0]
                yb = ys[:, 1, 0:b4, :, :, 1]
                nc.vector.scalar_tensor_tensor(
                    ypre,
                    _ap(st["xc2"][ti][:], [[4 * T, b4], [4, T], [2, 2]]),
                    dpc_sb[:, 0, ti, 0:1],
                    ys[:, 0, 0:b4, :, :, 0], ALU.mult, ALU.add)
                nc.vector.scalar_tensor_tensor(
                    yb,
                    _ap(st["xc2"][ti][:], [[4 * T, b4], [4, T], [2, 2]],
                        offset=1),
                    dpc_sb[:, 1, ti, 0:1],
                    ys[:, 0, 0:b4, :, :, 1], ALU.mult, ALU.add)
                # ypre += reverse_t(yb); then gate by silu(z)
                yb_rev = _ap(ys[:], [[4 * T, b4], [-4, T], [2, 2]],
                             offset=NB2 + 1 + 4 * (T - 1))
                nc.vector.tensor_add(ypre, ypre, yb_rev)
                # gate into a separate small tile so the out-proj matmul
                # does not pin the ys tile against the next scan (WAR)
                ypt = sp.tile([128, B4, T, 2], BF16, tag="ypt",
                              name=f"ypt{ti}_{ch}")
                yp_m = _ap(ys[:], [[2, b4 * T * 2]], offset=NB2)
                sz_i = _ap(st["sz"][ti][:], [[1, b4 * T * 2]])
                nc.vector.tensor_mul(
                    _ap(ypt[:], [[1, b4 * T * 2]]), yp_m, sz_i)
                # rhs iterated (b4, bpair, t) so ps_o columns are standard
                # (b, t) token order
                rhs = _ap(ypt[:], [[2 * T, b4], [1, 2], [2, T]])
                nc.tensor.matmul(ps_o[:, 0:cbt], wout_sb[:, ti, :], rhs,
                                 start=(ti == 0), stop=(ti == 1))

            tails[ch] = dict(ps_o=ps_o, uf=st["uf"])

        def back_tail(ch):
            b0, cb = OFF[ch], CBS[ch]
            cbt = cb * T
            tl = tails.pop(ch)
            o_sb = sp.tile([C, CBT], F32, tag="o_sb", name=f"o_sb{ch}")
            nc.scalar.copy(o_sb[:, 0:cbt], tl["ps_o"][:, 0:cbt])
            layernorm(o_sb[:, 0:cbt], ln2g_sb, ln2b_sb, o_sb[:, 0:cbt],
                      trivial=ln_trivial[1], cbt=cbt)
            nc.vector.tensor_add(o_sb[:, 0:cbt], o_sb[:, 0:cbt], tl["uf"])
            nc.sync.dma_start(out[:, b0:b0 + cb, :],
                              o_sb[:, 0:cbt].rearrange("p (b t) -> p b t",
                                                       t=T))

        for ch in range(NCHUNK):
            front(ch)
            if ch > 0:
                back(ch - 1)
            back_pre(ch)
            if ch > 1:
                back_tail(ch - 2)
        back(NCHUNK - 1)
        back_tail(NCHUNK - 2)
        back_tail(NCHUNK - 1)

    nc.finalize()
    return nc


def _prep(inputs):
    f = lambda k: np.ascontiguousarray(np.asarray(inputs[k], np.float32))
    bf = lambda a: np.ascontiguousarray(np.asarray(a, ml_dtypes.bfloat16))
    x = f("x")
    u_all = x.transpose(0, 2, 1, 3).reshape(B * N, T, C)
    u_pad = np.zeros((BSEQ, T, C), np.float32)
    u_pad[:B * N] = u_all
    xin = [bf(u_pad[i * BC:(i + 1) * BC].transpose(2, 0, 1))
           for i in range(NCORES)]

    A = -np.exp(f("A_log"))
    Ab = -np.exp(f("A_b_log"))
    assert np.allclose(A, A[0:1], rtol=1e-5), "A must be d-independent"
    assert np.allclose(Ab, A, rtol=1e-5), "A_b must equal A"
    a_pow = [float(v) for v in A[0]]

    w_in_t = f("in_proj_w").T                      # [C, 2*DI]
    w_in_x = w_in_t[:, :DI]                        # [C, DI]
    cw = np.stack([f("conv_w")[:, 0, :], f("conv_w_b")[:, 0, :]])  # [2,DI,4]
    # wconv[c, br, ti, k, d] = w_in_x[c, ti*128+d] * cw[br, ti*128+d, k]
    wconv = np.einsum('cd,bdk->bkcd', w_in_x, cw)  # [2, 4, C, DI]
    wconv = wconv.reshape(2, 4, C, 2, 128).transpose(2, 0, 3, 1, 4)
    cb = np.stack([f("conv_b"), f("conv_b_b")])[..., None]         # [2,DI,1]
    xw_ro = np.concatenate([f("xproj_w")[RK:], f("xproj_w")[:RK]])
    xw_ro_b = np.concatenate([f("xproj_w_b")[RK:], f("xproj_w_b")[:RK]])
    xwm = np.stack([xw_ro, xw_ro_b]).transpose(0, 2, 1)
    dtwm = np.stack([f("dtproj_w"), f("dtproj_w_b")]).transpose(0, 2, 1)
    dtbm = np.stack([f("dtproj_b"), f("dtproj_b_b")])[..., None]
    shared = {
        "w_z": bf(w_in_t[:, DI:]),
        "wconv": bf(wconv),
        "convb": np.ascontiguousarray(
            cb.reshape(2, 2, 128, 1).transpose(2, 0, 1, 3)),
        "xw": bf(xwm.reshape(2, 2, 128, 40).transpose(2, 0, 1, 3)),
        "dtw": bf(dtwm.transpose(1, 0, 2)),                        # [8,2,256]
        "dtb": np.ascontiguousarray(
            dtbm.reshape(2, 2, 128, 1).transpose(2, 0, 1, 3)),
        "dpc": np.ascontiguousarray(
            np.stack([f("Dp"), f("Dp_b")])[..., None]
            .reshape(2, 2, 128, 1).transpose(2, 0, 1, 3)),
        "wout": bf(
            f("out_proj_w").T.reshape(2, 128, 128).transpose(1, 0, 2)),
        "ln1g": f("ln1_g").reshape(C, 1),
        "ln1b": f("ln1_b").reshape(C, 1),
        "ln2g": f("ln2_g").reshape(C, 1),
        "ln2b": f("ln2_b").reshape(C, 1),
    }
    return xin, shared, a_pow


def _unshard(core_outs):
    y = np.stack(core_outs)                       # [8, C, BC, T]
    y = y.transpose(0, 2, 3, 1).reshape(BSEQ, T, C)[:B * N]
    return np.ascontiguousarray(
        y.reshape(B, N, T, C).transpose(0, 2, 1, 3))


_CACHE = {}


def kernel(_trace=False, **inputs):
    xin, shared, a_pow = _prep(inputs)
    if "prog" not in _CACHE:
        lt = (bool(np.all(inputs["ln1_g"] == 1) and np.all(inputs["ln1_b"] == 0)),
              bool(np.all(inputs["ln2_g"] == 1) and np.all(inputs["ln2_b"] == 0)))
        _CACHE["prog"] = build_program(a_pow, ln_trivial=lt)
    nc = _CACHE["prog"]
    in_maps = [dict(shared, xin=xin[i]) for i in range(NCORES)]
    res = run_bass_kernel_spmd(nc, in_maps, core_ids=list(range(NCORES)),
                               trace=_trace)
    out = _unshard([r["out"] for r in res.results])
    if _trace:
        kernel.last_results = res
    return out



# revision 20
# speedup vs baseline: 1.1867x; 1.1723x over previous
"""BiMamba (bimamba_type='v2') Trainium2 Bass kernel.

Data-parallel over the fused B*N=828 (padded to 896) sequence axis across 8
NeuronCores (112 sequences/core, 8 chunks of 14). Key design points:
  - SCAN4_ANT: custom DVE op (hand-built uOp tables, registered at runtime
    into the ant custom-op rows) runs the selective scan as four
    interleaved recurrences with states in the block-1/3 a/b result flops;
    the 2x_2p perf slot processes packed bf16 pairs at 2 elem/cycle —
    ~3.9x the stock tensor_tensor_scan (which pays a feedback bubble).
    Scan tensors live in a 4-chain layout [p, n, b4, t, bpair, branch]
    (chain = seq-pair half x branch), produced interleaved at the source.
  - depthwise causal conv folded into PE: per tap k, matmul of
    w_in_x[c,d]*conv_w[d,k] against shifted windows of the zero-padded LN1
    output (bwd branch via a reversed padded copy); front-end in bf16.
  - act-table patch: Exp/Ln resolve to natural_log_exp_and_others, killing
    the per-switch ACT_TABLE_LOAD ping-pong; PSUM->SBUF copies on ACT.
  - explicit front/back software pipelining (front(ch+1) emitted before
    back(ch)) with parity-buffered dA/brep tiles.
  - dt = ln(1+exp(.)) (no softplus table); LN rstd = exp(-0.5*ln(var+eps)).
"""

import numpy as np
import ml_dtypes

import concourse.bass as bass
import concourse.tile as tile
from concourse import bacc, mybir
from concourse.bass_utils import run_bass_kernel_spmd

# --- SCAN4_ANT: custom DVE op — 4-interleaved-chain multiply-add scan.
# Stream elements rotate over four independent recurrences (chain = k mod 4):
#   s[c] = d0[k]*s[c] + d1[k]; out[k] = s[c]
# States live in blocks 1/3's a/b result flops. The 1x slot issues 1
# elem/cycle (state re-read 4 cycles after write); the 2x_2p slot processes
# packed bf16 pairs at 2 elems/cycle, pairs alternating between chain groups
# (0,1) and (2,3) so each group's state is re-read 2 cycles after writing.
from dataclasses import dataclass as _dataclass

from concourse import dve_ops as _ops_mod
from concourse.dve_ops import _COMPILE_CACHE as _DVE_CACHE
from concourse.dve_spec import Spec as _Spec, Src0 as _Src0, Src1 as _Src1
from concourse.dve_uop import (
    ENABLE as _EN,
    AluInp as _AluInp,
    AluOp as _AluOp,
    DelayInp as _DelayInp,
    DveOpSpec as _DveOpSpec,
    InpSel as _InpSel,
    OutPath as _OutPath,
    OutSel as _OutSel,
    Trigger as _Trigger,
    UopConfig as _UopConfig,
)

_SCAN_NAME = "SCAN4_ANT"
_SCAN_ROW = 17  # rows 1..16 used by stock OPS; byte-36 row field < 0x20
_SEGDOT_NAME = "SEGDOT8_ANT"
_SEGDOT_ROW = 18


def _uop_1x(chain, init, nxt):
    u = _UopConfig()
    u.enable_input(_InpSel.SRC_0, 0)
    u.enable_input(_InpSel.SRC_1, 1)
    if init:
        u.enable_input(_InpSel.ZERO, 2)
    u.require_inp0 = _EN
    u.require_inp1 = _EN
    u.repeat_count = 1
    u.trigger = (_Trigger.SRC_TENSOR_DONE, _Trigger.COUNT, _Trigger.NONE)
    u.next_uop = (0, nxt, 0)
    u.enable_output(_OutSel.ALU_OUT, _OutPath.WR0_LO)
    mb, ab = (0, 1) if chain < 2 else (2, 3)
    flop_a = chain % 2 == 0
    state_src = _AluInp.PREV_DELAY_1 if init else (
        _AluInp.NEXT_ALU_OUT_A if flop_a else _AluInp.NEXT_ALU_OUT_B)
    for k in range(0, mb):
        u.datapath_config[k].pass_through_alu()
        u.datapath_config[k].pass_through_delay(0)
        if init:
            u.datapath_config[k].pass_through_delay(1)
    u.datapath_config[mb].enable_alu(_AluOp.MULTIPLY, _AluInp.PREV_ALU_OUT,
                                     state_src)
    u.datapath_config[mb].pass_through_delay(0)
    u.datapath_config[ab].enable_alu(_AluOp.ADD, _AluInp.PREV_ALU_OUT,
                                     _AluInp.PREV_DELAY_0)
    if flop_a:
        u.datapath_config[ab].alu_out_a_enable = _EN
    else:
        u.datapath_config[ab].alu_out_b_enable = _EN
    for k in range(ab + 1, 8):
        u.datapath_config[k].pass_through_alu()
    return u


def _uop_2x(group, init, nxt):
    u = _UopConfig()
    u.enable_input(_InpSel.SRC_0, 0)
    u.enable_input(_InpSel.SRC_1, 1)
    u.enable_input(_InpSel.SRC_0_HI, 2)
    u.enable_input(_InpSel.SRC_1_HI, 3)
    if init:
        u.enable_input(_InpSel.ZERO, 4)
    u.require_inp0 = _EN
    u.require_inp1 = _EN
    u.repeat_count = 1
    u.trigger = (_Trigger.SRC_TENSOR_DONE, _Trigger.COUNT, _Trigger.NONE)
    u.next_uop = (0, nxt, 0)
    u.enable_output(_OutSel.DELAY_3, _OutPath.WR0_LO)
    u.enable_output(_OutSel.ALU_OUT, _OutPath.WR0_HI)
    flop_a = group == 0
    st = _AluInp.PREV_DELAY_3 if init else (
        _AluInp.NEXT_ALU_OUT_A if flop_a else _AluInp.NEXT_ALU_OUT_B)
    b0 = u.datapath_config[0]
    b0.enable_alu(_AluOp.MULTIPLY, _AluInp.PREV_ALU_OUT, st)
    b0.pass_through_delay(0, 1, 2)
    if init:
        b0.pass_through_delay(3)
    b1 = u.datapath_config[1]
    b1.enable_alu(_AluOp.ADD, _AluInp.PREV_ALU_OUT, _AluInp.PREV_DELAY_0)
    if flop_a:
        b1.alu_out_a_enable = _EN
    else:
        b1.alu_out_b_enable = _EN
    b1.pass_through_delay(1, 2)
    if init:
        b1.pass_through_delay(3)
    b2 = u.datapath_config[2]
    b2.enable_alu(_AluOp.MULTIPLY, _AluInp.PREV_DELAY_1, st)
    b2.pass_through_delay(2)
    b2.enable_delay_from_src(_DelayInp.PREV_ALU_OUT, 3)
    b3 = u.datapath_config[3]
    b3.enable_alu(_AluOp.ADD, _AluInp.PREV_ALU_OUT, _AluInp.PREV_DELAY_2)
    if flop_a:
        b3.alu_out_a_enable = _EN
    else:
        b3.alu_out_b_enable = _EN
    b3.pass_through_delay(3)
    for k in range(4, 8):
        u.datapath_config[k].pass_through_alu()
        u.datapath_config[k].pass_through_delay(3)
    return u


# --- SEGDOT8_ANT: segmented dot-product. Stream = segments of 32 elements
# ([n=8 outer] x [quad=4 inner]); the 4 quad items are independent
# accumulators (chains); output = 4 values per segment:
#   out[seg, q] = sum_n in0[seg, n, q] * in1[seg, n, q]
# In 2x mode each cycle processes a packed quad-pair; groups (q0,q1)/(q2,q3)
# alternate. acc_lo lives in block-2 a/b flops (read by block-1's ADD via
# NEXT_ALU_OUT), acc_hi in block-5 a/b flops.


def _sd_uop_2x(group, init, emit, nxt):
    u = _UopConfig()
    u.enable_input(_InpSel.SRC_0, 0)
    u.enable_input(_InpSel.SRC_1, 1)
    u.enable_input(_InpSel.SRC_0_HI, 2)
    u.enable_input(_InpSel.SRC_1_HI, 3)
    if init:
        u.enable_input(_InpSel.ZERO, 4)
    u.require_inp0 = _EN
    u.require_inp1 = _EN
    u.repeat_count = 1
    u.trigger = (_Trigger.SRC_TENSOR_DONE, _Trigger.COUNT, _Trigger.NONE)
    u.next_uop = (0, nxt, 0)
    if emit:
        u.enable_output(_OutSel.DELAY_3, _OutPath.WR0_LO)
        u.enable_output(_OutSel.ALU_OUT, _OutPath.WR0_HI)
    flop_a = group == 0
    st = _AluInp.PREV_DELAY_3 if init else _AluInp.NEXT_ALU_OUT_A
    if not init and not flop_a:
        st = _AluInp.NEXT_ALU_OUT_B
    # b0: m_lo = h_lo * c_lo   (delay0 = c_lo consumed here)
    b0 = u.datapath_config[0]
    b0.enable_alu(_AluOp.MULTIPLY, _AluInp.PREV_ALU_OUT, _AluInp.PREV_DELAY_0)
    b0.pass_through_delay(1, 2)
    if init:
        b0.pass_through_delay(3)
    # b1: acc_lo' = m_lo + acc_lo (b2's a/b flop; ZERO via delay3 on init)
    b1 = u.datapath_config[1]
    b1.enable_alu(_AluOp.ADD, _AluInp.PREV_ALU_OUT, st)
    b1.pass_through_delay(1, 2)
    if init:
        b1.pass_through_delay(3)
    # b2: bypass acc_lo' into b2's a/b flop
    b2 = u.datapath_config[2]
    b2.pass_through_alu()
    if flop_a:
        b2.alu_out_a_enable = _EN
    else:
        b2.alu_out_b_enable = _EN
    b2.pass_through_delay(1, 2)
    if init:
        b2.pass_through_delay(3)
    # b3: m_hi = h_hi * c_hi; on emit also stage acc_lo' into delay3
    b3 = u.datapath_config[3]
    b3.enable_alu(_AluOp.MULTIPLY, _AluInp.PREV_DELAY_1, _AluInp.PREV_DELAY_2)
    if emit:
        b3.enable_delay_from_src(_DelayInp.PREV_ALU_OUT, 3)
    elif init:
        b3.pass_through_delay(3)
    # b4: acc_hi' = m_hi + acc_hi (b5's a/b flop)
    b4 = u.datapath_config[4]
    st_hi = _AluInp.PREV_DELAY_3 if init else _AluInp.NEXT_ALU_OUT_A
    if not init and not flop_a:
        st_hi = _AluInp.NEXT_ALU_OUT_B
    b4.enable_alu(_AluOp.ADD, _AluInp.PREV_ALU_OUT, st_hi)
    if emit:
        b4.pass_through_delay(3)
    # b5: bypass acc_hi' into b5's a/b flop
    b5 = u.datapath_config[5]
    b5.pass_through_alu()
    if flop_a:
        b5.alu_out_a_enable = _EN
    else:
        b5.alu_out_b_enable = _EN
    if emit:
        b5.pass_through_delay(3)
    for k in range(6, 8):
        u.datapath_config[k].pass_through_alu()
        if emit:
            u.datapath_config[k].pass_through_delay(3)
    return u


def _sd_uop_1x(chain, init, emit, nxt):
    u = _UopConfig()
    u.enable_input(_InpSel.SRC_0, 0)
    u.enable_input(_InpSel.SRC_1, 1)
    if init:
        u.enable_input(_InpSel.ZERO, 2)
    u.require_inp0 = _EN
    u.require_inp1 = _EN
    u.repeat_count = 1
    u.trigger = (_Trigger.SRC_TENSOR_DONE, _Trigger.COUNT, _Trigger.NONE)
    u.next_uop = (0, nxt, 0)
    if emit:
        u.enable_output(_OutSel.ALU_OUT, _OutPath.WR0_LO)
    flop_a = chain % 2 == 0
    ab = 1 if chain < 2 else 3         # ADD block; store block = ab+1
    st = _AluInp.PREV_DELAY_1 if init else (
        _AluInp.NEXT_ALU_OUT_A if flop_a else _AluInp.NEXT_ALU_OUT_B)
    b0 = u.datapath_config[0]
    b0.enable_alu(_AluOp.MULTIPLY, _AluInp.PREV_ALU_OUT, _AluInp.PREV_DELAY_0)
    if init:
        b0.pass_through_delay(1)
    for k in range(1, ab):
        u.datapath_config[k].pass_through_alu()
        if init:
            u.datapath_config[k].pass_through_delay(1)
    u.datapath_config[ab].enable_alu(_AluOp.ADD, _AluInp.PREV_ALU_OUT, st)
    bs = u.datapath_config[ab + 1]
    bs.pass_through_alu()
    if flop_a:
        bs.alu_out_a_enable = _EN
    else:
        bs.alu_out_b_enable = _EN
    for k in range(ab + 2, 8):
        u.datapath_config[k].pass_through_alu()
    return u


@_dataclass(frozen=True)
class _ShimSpec:
    accum: object = None


class _ScanOp:
    name = _SCAN_NAME
    subdim = False
    spec = _ShimSpec()
    perf_en: dict = {}

    def compile(self, ver):
        key = (self.name, ver)
        if key not in _DVE_CACHE:
            uops = [
                _uop_1x(0, True, 1), _uop_1x(1, True, 2),
                _uop_1x(2, True, 3), _uop_1x(3, True, 4),
                _uop_1x(0, False, 5), _uop_1x(1, False, 6),
                _uop_1x(2, False, 7), _uop_1x(3, False, 4),
            ]
            u2 = [
                _uop_2x(0, True, 1), _uop_2x(1, True, 2),
                _uop_2x(0, False, 3), _uop_2x(1, False, 2),
                _uop_2x(0, False, 3), _uop_2x(1, False, 2),
                _uop_2x(0, False, 3), _uop_2x(1, False, 2),
            ]
            u2p = [
                _uop_2x(0, True, 1), _uop_2x(1, True, 2),
                _uop_2x(0, False, 3), _uop_2x(1, False, 2),
                _uop_2x(0, False, 3), _uop_2x(1, False, 2),
                _uop_2x(0, False, 3), _uop_2x(1, False, 2),
            ]
            _DVE_CACHE[key] = _DveOpSpec(
                name=self.name, opcode=_SCAN_ROW, uops=uops,
                uops_2x=u2, uops_2x_2p=u2p, perf_max=2, rd1_en=True)
        return _DVE_CACHE[key]


_SCAN4 = _ScanOp()


class _SegDotOp:
    name = _SEGDOT_NAME
    subdim = False
    spec = _ShimSpec()
    perf_en: dict = {}

    def compile(self, ver):
        key = (self.name, ver)
        if key not in _DVE_CACHE:
            def chain2x():
                us = [_sd_uop_2x(0, True, False, 2)]     # entry
                us.append(_sd_uop_2x(0, True, False, 2))  # loop initA
                us.append(_sd_uop_2x(1, True, False, 3))  # initB
                for j in range(6):
                    us.append(_sd_uop_2x(0, False, False, 4 + 2 * j))
                    us.append(_sd_uop_2x(1, False, False, 5 + 2 * j))
                us.append(_sd_uop_2x(0, False, True, 16))
                us.append(_sd_uop_2x(1, False, True, 1))
                return us

            # The op's APs statically satisfy the 2X_1PORT trigger
            # conditions (2B dtype, unit inner step, 4B alignment), so the
            # REGULAR slot is never reached — fill it with the 2x chain to
            # stay inside the 256-entry control table. uops_2x_2p=None
            # reuses the 2X_1PORT continuation slots.
            _DVE_CACHE[key] = _DveOpSpec(
                name=self.name, opcode=_SEGDOT_ROW, uops=chain2x(),
                uops_2x=chain2x(), uops_2x_2p=None, perf_max=2,
                rd1_en=True)
        return _DVE_CACHE[key]


_SEGDOT = _SegDotOp()


def _segdot_register():
    if _SEGDOT_NAME in _ops_mod._SUB_OPCODE_FOR_NAME:
        return
    _ops_mod._SUB_OPCODE_FOR_NAME[_SEGDOT_NAME] = _SEGDOT_ROW
    _ops_mod.OPS.append(_SEGDOT)
    _ops_mod.CUSTOM_DVE_SPECS[_SEGDOT_NAME] = _Spec(
        body=_Src0 * _Src1,
        reference=lambda in0, in1, s0, s1, imm2: in0 * in1,
    )


def _segdot_emit(nc, out, in0, in1):
    _segdot_register()
    from concourse import bass_isa
    from concourse.bass_utils import dve_ver_for

    v = nc.vector
    if _SEGDOT.name not in v.bass.m.ant_custom_dve_ops:
        v.bass.m.ant_custom_dve_ops = sorted(
            {*v.bass.m.ant_custom_dve_ops, _SEGDOT.name})
    _SEGDOT.compile(dve_ver_for(v.bass.trn_type))
    shape = bass_isa.CustomDveShape.TTSS
    isa_opcode = v.bass.isa.Opcode[
        f"NEURON_ISA_TPB_OPCODE_CUSTOM_DVE_ANT_{shape.slot()}"].value
    imm = mybir.ImmediateValue(dtype=mybir.dt.float32, value=0.0)
    inst = bass_isa.InstCustomDveAnt(
        name=v.bass.get_next_instruction_name(),
        op_name=_SEGDOT.name,
        rd1_en=True,
        subdim=0,
        imm2=0.0,
        shape=shape,
        row=_SEGDOT_ROW,
        isa_opcode=isa_opcode,
        ins=[v.lower_ap(in0, for_isa=True),
             v.lower_ap(in1, for_isa=True), imm, imm],
        outs=[v.lower_ap(out, for_isa=True)],
    )
    inst.perf_max = 2
    return v.add_instruction(inst)


def _scan4_register():
    if _SCAN_NAME in _ops_mod._SUB_OPCODE_FOR_NAME:
        return
    _ops_mod._SUB_OPCODE_FOR_NAME[_SCAN_NAME] = _SCAN_ROW
    _ops_mod.OPS.append(_SCAN4)
    _ops_mod.CUSTOM_DVE_SPECS[_SCAN_NAME] = _Spec(
        body=_Src0 * _Src1,
        reference=lambda in0, in1, s0, s1, imm2: in0 * in1,
    )


def _scan4_emit(nc, out, d0, d1):
    _scan4_register()
    from concourse import bass_isa
    from concourse.bass_utils import dve_ver_for

    v = nc.vector
    if _SCAN4.name not in v.bass.m.ant_custom_dve_ops:
        v.bass.m.ant_custom_dve_ops = sorted(
            {*v.bass.m.ant_custom_dve_ops, _SCAN4.name})
    _SCAN4.compile(dve_ver_for(v.bass.trn_type))
    shape = bass_isa.CustomDveShape.TTSS
    isa_opcode = v.bass.isa.Opcode[
        f"NEURON_ISA_TPB_OPCODE_CUSTOM_DVE_ANT_{shape.slot()}"].value
    imm = mybir.ImmediateValue(dtype=mybir.dt.float32, value=0.0)
    inst = bass_isa.InstCustomDveAnt(
        name=v.bass.get_next_instruction_name(),
        op_name=_SCAN4.name,
        rd1_en=True,
        subdim=0,
        imm2=0.0,
        shape=shape,
        row=_SCAN_ROW,
        isa_opcode=isa_opcode,
        ins=[v.lower_ap(d0, for_isa=True),
             v.lower_ap(d1, for_isa=True), imm, imm],
        outs=[v.lower_ap(out, for_isa=True)],
    )
    inst.perf_max = 2
    return v.add_instruction(inst)



F32 = mybir.dt.float32
BF16 = mybir.dt.bfloat16
AF = mybir.ActivationFunctionType
ALU = mybir.AluOpType

B, T, N, C = 4, 24, 207, 128
DI = 256
DS = 16
RK = 8
EPS = 1e-5
NCORES = 8
BSEQ = 832
BC = BSEQ // NCORES          # 104 sequences per core (828 real + 4 pad)
NCHUNK = 8
CBS = (14, 14, 14, 14, 14, 14, 10, 10)   # per-chunk seqs (must be even)
OFF = tuple(sum(CBS[:i]) for i in range(NCHUNK))
CB = max(CBS)                # tile-shape maximum
B4 = CB // 2                 # sequence pairs (scan chain interleave)
CBT = CB * T                 # tokens per max chunk
TP = T + 3                   # left-padded time for causal conv windows

# --- act-table patch: make the set chooser pick natural_log_exp_and_others
# for both Exp and Ln (otherwise it alternates exp_and_others/natural_log
# and reloads tables on every switch).
import concourse.bacc as _bacc_mod
from concourse.hw_specs import get_activation_tables as _orig_gat


def _patched_gat(arch):
    t = dict(_orig_gat(arch))
    for nm, drop in (("exp_and_others", AF.Exp), ("exp_and_friends", AF.Exp),
                     ("natural_log", AF.Ln)):
        if nm in t:
            t[nm] = set(t[nm]) - {drop}
    return t


_bacc_mod.get_activation_tables = _patched_gat


def _pbcast(ap, parts=128):
    a = [[0, parts]] + [list(x) for x in ap.ap]
    return bass.AP(tensor=ap.tensor, offset=ap.offset, ap=a)


def _rev_t(ap):
    a = [list(x) for x in ap.ap]
    st, ct = a[-1]
    off = ap.offset + st * (ct - 1)
    a[-1] = [-st, ct]
    return bass.AP(tensor=ap.tensor, offset=off, ap=a)


def _zstride(ap, dim, count):
    a = [list(x) for x in ap.ap]
    a.insert(1 + dim, [0, count])
    return bass.AP(tensor=ap.tensor, offset=ap.offset, ap=a)


def _ap(base, dims, offset=0):
    """AP over base's tensor: keep base's partition dim, explicit free dims
    [[stride, count], ...], extra element offset."""
    return bass.AP(tensor=base.tensor, offset=base.offset + offset,
                   ap=[list(base.ap[0])] + [list(d) for d in dims])


def build_program(a_pow, ln_trivial=(False, False)):
    nc = bacc.Bacc("TRN2", target_bir_lowering=False, debug=False,
                   enable_asserts=False, num_devices=NCORES)

    def din(name, shape, dt=F32):
        return nc.dram_tensor(name, shape, dt, kind="ExternalInput").ap()

    xin = din("xin", [C, BC, T], BF16)
    w_z = din("w_z", [C, 2 * C], BF16)            # z half of in_proj
    wconv = din("wconv", [C, 2, 2, 4, C], BF16)   # [c, br, ti, k, d]
    convb = din("convb", [128, 2, 2, 1])
    xw = din("xw", [128, 2, 2, 40], BF16)
    dtw = din("dtw", [RK, 2, DI], BF16)
    dtb = din("dtb", [128, 2, 2, 1])
    dpc = din("dpc", [128, 2, 2, 1])
    wout = din("wout", [128, 2, C], BF16)
    ln1g = din("ln1g", [C, 1])
    ln1b = din("ln1b", [C, 1])
    ln2g = din("ln2g", [C, 1])
    ln2b = din("ln2b", [C, 1])
    out = nc.dram_tensor("out", [C, BC, T], F32, kind="ExternalOutput").ap()

    with tile.TileContext(nc) as tc, \
         tc.tile_pool(name="weights", bufs=1) as wp, \
         tc.tile_pool(name="small", bufs=2) as sp, \
         tc.tile_pool(name="stats", bufs=2) as stp, \
         tc.tile_pool(name="dbu", bufs=1) as bp, \
         tc.tile_pool(name="brep", bufs=2) as brp, \
         tc.tile_pool(name="crep", bufs=1) as crp, \
         tc.tile_pool(name="dram", bufs=2, space="DRAM") as drp, \
         tc.tile_pool(name="psA", bufs=2, space="PSUM") as psA, \
         tc.tile_pool(name="psCv", bufs=2, space="PSUM") as psCv, \
         tc.tile_pool(name="psB", bufs=2, space="PSUM") as psB, \
         tc.tile_pool(name="psO", bufs=2, space="PSUM") as psO:

        pre_u = {}
        for ch0 in range(2):
            t = sp.tile([C, CB, T], BF16, tag="u", name=f"u{ch0}", bufs=3)
            nc.sync.dma_start(t[:, 0:CBS[ch0], :],
                              xin[:, OFF[ch0]:OFF[ch0] + CBS[ch0], :])
            pre_u[ch0] = t

        def load_w(name, ap_src, shape, dt=F32):
            t = wp.tile(shape, dt, tag=name, name=name)
            nc.sync.dma_start(t[:], ap_src)
            return t

        w_z_sb = load_w("w_z", w_z, [C, 2 * C], BF16)
        # wconv is 512KB — split the load across 8 DMAs so it spreads over
        # parallel queues instead of serializing ~23us on one
        wconv_sb = wp.tile([C, 2, 2, 4, C], BF16, tag="wconv", name="wconv")
        for _br in range(2):
            for _ti in range(2):
                for _kh in range(2):
                    nc.sync.dma_start(
                        wconv_sb[:, _br, _ti, 2 * _kh:2 * _kh + 2, :],
                        wconv[:, _br, _ti, 2 * _kh:2 * _kh + 2, :])
        convb_sb = load_w("convb", convb, [128, 2, 2, 1])
        xw_sb = load_w("xw", xw, [128, 2, 2, 40], BF16)
        dtw_sb = load_w("dtw", dtw, [RK, 2, DI], BF16)
        dtb_sb = load_w("dtb", dtb, [128, 2, 2, 1])
        dpc_sb = load_w("dpc", dpc, [128, 2, 2, 1])
        wout_sb = load_w("wout", wout, [128, 2, C], BF16)
        ones_bf = wp.tile([C, 1], BF16, tag="ones_bf")
        nc.vector.memset(ones_bf[:], 1.0)
        ln1g_sb = load_w("ln1g", ln1g, [C, 1])
        ln1b_sb = load_w("ln1b", ln1b, [C, 1])
        ln2g_sb = load_w("ln2g", ln2g, [C, 1])
        ln2b_sb = load_w("ln2b", ln2b, [C, 1])
        ones_sb = wp.tile([C, 1], F32, tag="ones")
        nc.vector.memset(ones_sb[:], 1.0)
        eps_sb = wp.tile([C, 1], F32, tag="eps")
        nc.vector.memset(eps_sb[:], EPS)
        ones_row = wp.tile([1, C], F32, tag="ones_row")
        nc.vector.memset(ones_row[:], 1.0)

        # persistent padded LN1 outputs (fwd + reversed), 2 parities
        hlnp = [wp.tile([C, CB, TP], BF16, tag=f"hlnp{i}", name=f"hlnp{i}")
                for i in range(2)]
        hlnr = [wp.tile([C, CB, TP], BF16, tag=f"hlnr{i}", name=f"hlnr{i}")
                for i in range(2)]
        for tl in hlnp + hlnr:
            nc.gpsimd.memset(tl[:, :, 0:3], 0.0)

        # persistent dA tiles in 4-chain interleaved layout
        # [p, n, b4, t, bpair, a]: ti=0 double-buffered (exps in front),
        # ti=1 single (exps at back start). t=0 column zero = segment reset.
        dA0 = [wp.tile([128, DS, B4, T, 2, 2], BF16, tag=f"dA0_{i}",
                       name=f"dA0_{i}") for i in range(2)]
        dA1 = wp.tile([128, DS, B4, T, 2, 2], BF16, tag="dA1", name="dA1")
        for tl in dA0 + [dA1]:
            nc.gpsimd.memset(tl[:, :, :, 0:1, :, :], 0.0)

        def layernorm(src_f32, g_sb, b_sb, dst, trivial=False, cbt=CBT):
            """LN over channel (partition) dim of src [C, CBT] -> dst view."""
            sq = sp.tile([C, CBT], BF16, tag="ln_sq", bufs=1)
            nc.scalar.activation(sq[:, 0:cbt], src_f32, AF.Square)
            ps_s = psA.tile([128, CBT], F32, tag="pm", name="ps_s")
            ps_q = psA.tile([128, CBT], F32, tag="pm", name="ps_q")
            ones_like = (ones_bf if src_f32.tensor.dtype == BF16
                         else ones_sb)
            nc.tensor.matmul(ps_s[0:1, 0:cbt], ones_like[:], src_f32,
                             start=True, stop=True)
            nc.tensor.matmul(ps_q[0:1, 0:cbt], ones_bf[:], sq[:, 0:cbt],
                             start=True, stop=True)
            mean = stp.tile([1, CBT], F32, tag="mean")
            nc.scalar.mul(mean[:, 0:cbt], ps_s[0:1, 0:cbt], 1.0 / C)
            var = stp.tile([1, CBT], F32, tag="var")
            nc.scalar.mul(var[:, 0:cbt], ps_q[0:1, 0:cbt], 1.0 / C)
            m2 = stp.tile([1, CBT], F32, tag="m2")
            nc.scalar.square(m2[:, 0:cbt], mean[:, 0:cbt])
            nc.vector.tensor_sub(var[:, 0:cbt], var[:, 0:cbt], m2[:, 0:cbt])
            # rstd = (var+eps)^-0.5 = exp(-0.5*ln(var+eps))
            nc.scalar.activation(var[:, 0:cbt], var[:, 0:cbt], AF.Ln,
                                 bias=eps_sb[0:1, 0:1])
            nc.scalar.activation(var[:, 0:cbt], var[:, 0:cbt], AF.Exp,
                                 scale=-0.5)
            mean_r = psB.tile([C, CBT], F32, tag="pb", name="mean_r")
            nc.tensor.matmul(mean_r[:, 0:cbt], ones_row[:], mean[:, 0:cbt],
                             start=True, stop=True)
            rstd_r = psB.tile([C, CBT], F32, tag="pb", name="rstd_r")
            nc.tensor.matmul(rstd_r[:, 0:cbt], ones_row[:], var[:, 0:cbt],
                             start=True, stop=True)
            tmp = sp.tile([C, CBT], BF16, tag="ln_tmp", bufs=1)
            nc.vector.tensor_sub(tmp[:, 0:cbt], src_f32, mean_r[:, 0:cbt])
            if trivial:
                # g==1, b==0: write the normalize directly to dst
                rv = rstd_r[:, 0:cbt]
                if len(dst.shape) == 3:
                    tv = tmp[:, 0:cbt].rearrange("p (b t) -> p b t",
                                                 t=dst.shape[2])
                    rv = rv.rearrange("p (b t) -> p b t", t=dst.shape[2])
                else:
                    tv = tmp[:, 0:cbt]
                nc.vector.tensor_mul(dst, tv, rv)
                return
            nc.vector.tensor_mul(tmp[:, 0:cbt], tmp[:, 0:cbt],
                                 rstd_r[:, 0:cbt])
            tv = tmp[:, 0:cbt]
            if len(dst.shape) == 3:
                tv = tv.rearrange("p (b t) -> p b t", t=dst.shape[2])
            nc.vector.tensor_scalar(dst, tv, g_sb[:, 0:1], b_sb[:, 0:1],
                                    ALU.mult, ALU.add)

        state = {}
        tails = {}

        def front(ch):
            par = ch % 2
            b0, cb = OFF[ch], CBS[ch]
            b4, cbt = cb // 2, cb * T
            if ch in pre_u:
                u = pre_u.pop(ch)
            else:
                u = sp.tile([C, CB, T], BF16, tag="u", name=f"u{ch}",
                            bufs=3)
                nc.sync.dma_start(u[:, 0:cb, :], xin[:, b0:b0 + cb, :])
            uf = u[:, 0:cb, :].rearrange("p b t -> p (b t)")

            hp, hr = hlnp[par], hlnr[par]
            layernorm(uf, ln1g_sb, ln1b_sb, hp[:, 0:cb, 3:TP],
                      trivial=ln_trivial[0], cbt=cbt)
            # reversed copy for the bwd-branch conv windows
            nc.scalar.copy(hr[:, 0:cb, 3:TP], _rev_t(hp[:, 0:cb, 3:TP]))

            # z half + silu gate
            sz = [sp.tile([128, B4, T, 2], BF16, tag=f"sz{ti}",
                          name=f"sz{ti}_{ch}") for ti in range(2)]
            for ti in range(2):
                ps_z = psA.tile([128, CBT], F32, tag="pm", name=f"ps_z{ti}")
                nc.tensor.matmul(ps_z[:, 0:cbt],
                                 w_z_sb[:, ti * 128:(ti + 1) * 128],
                                 hp[:, 0:cb, 3:TP], start=True, stop=True)
                nc.scalar.activation(
                    sz[ti][:, 0:b4, :, :],
                    _ap(ps_z[:], [[2 * T, b4], [1, T], [T, 2]]),
                    AF.Silu)

            # conv via shifted-window matmuls (weights pre-folded w/
            # in_proj); xc2 written in 4-chain layout [p, b4, t, bpair, a]
            xc2 = [sp.tile([128, B4, T, 2, 2], BF16, tag=f"xc{ti}",
                           name=f"xc{ti}_{ch}") for ti in range(2)]
            for ti in range(2):
                for br in range(2):
                    src = hp if br == 0 else hr
                    ps_c = psCv.tile([128, CB, T], F32, tag="pc")
                    for j, k in enumerate((3, 2, 1, 0)):
                        nc.tensor.matmul(ps_c[:, 0:cb, :],
                                         wconv_sb[:, br, ti, k, :],
                                         src[:, 0:cb, k:k + T],
                                         start=(j == 0), stop=(j == 3))
                    nc.scalar.activation(
                        xc2[ti][:, 0:b4, :, :, br],
                        _ap(ps_c[:], [[2 * T, b4], [1, T], [T, 2]]),
                        AF.Silu, bias=convb_sb[:, br, ti, 0:1])

            # xproj -> x_dbl [40, CBT] per branch; B/C staged branch-
            # interleaved [n, b, t, a] via cheap strided ACT copies so the
            # DRAM round-trip DMAs stay fully contiguous.
            dtraw = [None, None]
            bc2 = stp.tile([32, B4, T, 2, 2], BF16, tag="bc2",
                           name=f"bc2_{ch}")
            for br in range(2):
                ps_xd = psA.tile([128, CBT], F32, tag="pm", name=f"ps_xd{br}")
                for ti in range(2):
                    nc.tensor.matmul(ps_xd[0:40, 0:cbt], xw_sb[:, br, ti, :],
                                     _ap(xc2[ti][:],
                                         [[4 * T, b4], [4, T], [2, 2]],
                                         offset=br),
                                     start=(ti == 0), stop=(ti == 1))
                nc.scalar.copy(bc2[:, 0:b4, :, :, br],
                               ps_xd[0:32, 0:cbt].rearrange(
                                   "p (b t x) -> p b t x", t=T, x=2))
                dtraw[br] = stp.tile([RK, CBT], BF16, tag=f"dtraw{br}",
                                     name=f"dtraw{br}_{ch}")
                nc.scalar.copy(dtraw[br][:, 0:cbt], ps_xd[32:40, 0:cbt])

            # B/C broadcast staging (DRAM round-trip); brep+crep loads here
            # (both bufs=2). b1d write + brep broadcast are issued BEFORE
            # the c1d transpose (whose 8-byte-granule descriptors would
            # otherwise delay them in the DMA queues). c1d is written in
            # the segdot layout [half, b4, t, n8, bpair, branch].
            b1d = drp.tile([DS, B4, T, 2, 2], BF16, tag="b1d")
            c1d = drp.tile([2, B4, T, 8, 2, 2], BF16, tag="c1d")
            nc.sync.dma_start(b1d[:, 0:b4], bc2[0:DS, 0:b4])
            NB = B4 * T * 4
            brep = brp.tile([128, DS * CBT * 2], BF16, tag="brep")
            b1f = b1d[:].rearrange("n b t x a -> (n b t x a)")
            nc.sync.dma_start(
                _ap(brep[:], [[NB, DS], [1, b4 * T * 4]]),
                _pbcast(bass.AP(tensor=b1f.tensor, offset=b1f.offset,
                                ap=[[NB, DS], [1, b4 * T * 4]])))
            for hf_ in range(2):
                dst = bass.AP(
                    tensor=c1d[:].tensor,
                    offset=c1d[:].offset + hf_ * (B4 * T * 32),
                    ap=[[4, 8], [32, b4 * T], [1, 4]])
                nc.sync.dma_start(dst, bc2[DS + 8 * hf_:DS + 8 * hf_ + 8,
                                           0:b4])

            # dtproj; dt = ln(1 + exp(x + bias)); dt2/du2 in the 4-chain
            # layout [p, b4, t, bpair, a]
            dt2 = [sp.tile([128, B4, T, 2, 2], BF16, tag=f"dt{ti}",
                           name=f"dt{ti}_{ch}", bufs=1 if ti == 0 else 2)
                   for ti in range(2)]
            for br in range(2):
                for ti in range(2):
                    ps_dt = psA.tile([128, CBT], F32, tag="pm",
                                     name=f"ps_dt{br}{ti}")
                    nc.tensor.matmul(ps_dt[:, 0:cbt],
                                     dtw_sb[:, br, ti * 128:(ti + 1) * 128],
                                     dtraw[br][:, 0:cbt],
                                     start=True, stop=True)
                    slab = dt2[ti][:, 0:b4, :, :, br]
                    nc.scalar.activation(
                        slab,
                        ps_dt[:, 0:cbt].rearrange("p (b t x) -> p b t x",
                                                  t=T, x=2),
                        AF.Exp, bias=dtb_sb[:, br, ti, 0:1])
                    nc.scalar.activation(slab, slab, AF.Ln, bias=1.0)

            # du = dt * xc (bf16, 4-chain layout; xc read strided)
            du2 = [sp.tile([128, B4, T, 2, 2], BF16, tag=f"du{ti}",
                           name=f"du{ti}_{ch}") for ti in range(2)]
            for ti in range(2):
                nc.vector.tensor_mul(du2[ti][:, 0:b4], dt2[ti][:, 0:b4],
                                     xc2[ti][:, 0:b4])

            # dA for ti=0 (parity tile); exp over t in [1, T)
            for n in range(DS):
                nc.scalar.activation(dA0[par][:, n, 0:b4, 1:T, :, :],
                                     dt2[0][:, 0:b4, 1:T, :, :],
                                     AF.Exp, scale=float(a_pow[n]))

            state[ch] = dict(u=u, uf=uf, sz=sz, xc2=xc2, du2=du2, dt2=dt2,
                             b1d=b1d, c1d=c1d, brep=brep)

        def back_pre(ch):
            """crep broadcast + dA1 exps: emitted right after back(ch-1)
            (WAR on dA1/crep) and before front(ch+1), so they are queued
            ahead of the next chunk's ACT/DMA work."""
            b4 = CBS[ch] // 2
            st = state[ch]
            # crep in segdot layout [half, b4, t, n8, bpair, branch]; both
            # sides fully contiguous per half.
            HSEG = B4 * T * 32
            useg = b4 * T * 32
            crep = crp.tile([128, 2 * HSEG], BF16, tag="crep")
            c1f = st["c1d"][:].rearrange("h b t n x a -> (h b t n x a)")
            for hf_ in range(2):
                nc.sync.dma_start(
                    _ap(crep[:], [[1, useg]], offset=hf_ * HSEG),
                    _pbcast(bass.AP(tensor=c1f.tensor,
                                    offset=c1f.offset + hf_ * HSEG,
                                    ap=[[1, useg]])))
            st["crep"] = crep

        def back(ch):
            par = ch % 2
            b0, cb = OFF[ch], CBS[ch]
            b4, cbt = cb // 2, cb * T
            NB = B4 * T * 4            # full per-n block (tile layout)
            UB = b4 * T * 4            # used span per n block
            HSEG = B4 * T * 32
            st = state.pop(ch)
            brepf = st["brep"][:]
            crepf = st["crep"][:]

            # dA for ti=1 (single tile; ACT runs during dBu_0/scan_0)
            for n in range(DS):
                nc.scalar.activation(dA1[:, n, 0:b4, 1:T, :, :],
                                     st["dt2"][1][:, 0:b4, 1:T, :, :],
                                     AF.Exp, scale=float(a_pow[n]))

            ps_o = psO.tile([C, CBT], F32, tag="po", name=f"ps_o{ch}")
            HN = DS // 2
            HSZ = HN * NB
            for ti in range(2):
                du4 = _ap(st["du2"][ti][:], [[4 * T, b4], [1, T * 4]])
                dA = dA0[par] if ti == 0 else dA1
                # h in segdot layout [half, b4, t, n8, bpair, branch]
                h = bp.tile([128, 2, B4, T, 8, 2, 2], BF16, tag="h")
                for hf_ in range(2):
                    dBu = bp.tile([128, HN, B4, T, 2, 2], BF16, tag="dBu")
                    # write compacted (n-stride = UB) so the scan's src1 is
                    # a single contiguous free dim (TTSS src1 must be 1D)
                    nc.vector.tensor_mul(
                        _ap(dBu[:], [[UB, HN], [4 * T, b4], [1, T * 4]]),
                        _zstride(du4, 0, HN),
                        _ap(brepf, [[NB, HN], [4 * T, b4], [1, T * 4]],
                            offset=hf_ * HSZ))
                    for n8 in range(HN):
                        _scan4_emit(
                            nc,
                            _ap(h[:], [[32, b4 * T], [1, 4]],
                                offset=hf_ * HSEG + 4 * n8),
                            _ap(dA[:], [[4, b4 * T], [1, 4]],
                                offset=(hf_ * HN + n8) * NB),
                            _ap(dBu[:], [[1, UB]], offset=n8 * UB))
                # segmented dot with C: ys[half, b4, t, bp, br]
                ys = bp.tile([128, 2, B4, T, 2, 2], BF16, tag="ys")
                for hf_ in range(2):
                    _segdot_emit(
                        nc,
                        _ap(ys[:], [[4, b4 * T], [1, 4]],
                            offset=hf_ * (B4 * T * 4)),
                        _ap(h[:], [[32, b4 * T], [1, 32]],
                            offset=hf_ * HSEG),
                        _ap(crepf, [[32, b4 * T], [1, 32]],
                            offset=hf_ * HSEG))
                yv = _ap(ys[:], [[1, b4 * T * 4]])
                nc.vector.tensor_add(
                    yv, yv, _ap(ys[:], [[1, b4 * T * 4]], offset=B4 * T * 4))
                # stage ypre/yb in the (now dead) ys half-1 slab
                NB2 = B4 * T * 4
                ypre = ys[:, 1, 0:b4, :, :, 0]
                yb = ys[:, 1, 0:b4, :, :, 1]
                nc.vector.scalar_tensor_tensor(
                    ypre,
                    _ap(st["xc2"][ti][:], [[4 * T, b4], [4, T], [2, 2]]),
                    dpc_sb[:, 0, ti, 0:1],
                    ys[:, 0, 0:b4, :, :, 0], ALU.mult, ALU.add)
                nc.vector.scalar_tensor_tensor(
                    yb,
                    _ap(st["xc2"][ti][:], [[4 * T, b4], [4, T], [2, 2]],
                        offset=1),
                    dpc_sb[:, 1, ti, 0:1],
                    ys[:, 0, 0:b4, :, :, 1], ALU.mult, ALU.add)
                # ypre += reverse_t(yb); then gate by silu(z)
                yb_rev = _ap(ys[:], [[4 * T, b4], [-4, T], [2, 2]],
                             offset=NB2 + 1 + 4 * (T - 1))
                nc.vector.tensor_add(ypre, ypre, yb_rev)
                # gate into a separate small tile so the out-proj matmul
                # does not pin the ys tile against the next scan (WAR)
                ypt = sp.tile([128, B4, T, 2], BF16, tag="ypt",
                              name=f"ypt{ti}_{ch}")
                yp_m = _ap(ys[:], [[2, b4 * T * 2]], offset=NB2)
                sz_i = _ap(st["sz"][ti][:], [[1, b4 * T * 2]])
                nc.vector.tensor_mul(
                    _ap(ypt[:], [[1, b4 * T * 2]]), yp_m, sz_i)
                # rhs iterated (b4, bpair, t) so ps_o columns are standard
                # (b, t) token order
                rhs = _ap(ypt[:], [[2 * T, b4], [1, 2], [2, T]])
                nc.tensor.matmul(ps_o[:, 0:cbt], wout_sb[:, ti, :], rhs,
                                 start=(ti == 0), stop=(ti == 1))

            tails[ch] = dict(ps_o=ps_o, uf=st["uf"])

        def back_tail(ch):
            b0, cb = OFF[ch], CBS[ch]
            cbt = cb * T
            tl = tails.pop(ch)
            o_sb = sp.tile([C, CBT], F32, tag="o_sb", name=f"o_sb{ch}")
            nc.scalar.copy(o_sb[:, 0:cbt], tl["ps_o"][:, 0:cbt])
            layernorm(o_sb[:, 0:cbt], ln2g_sb, ln2b_sb, o_sb[:, 0:cbt],
                      trivial=ln_trivial[1], cbt=cbt)
            nc.vector.tensor_add(o_sb[:, 0:cbt], o_sb[:, 0:cbt], tl["uf"])
            nc.sync.dma_start(out[:, b0:b0 + cb, :],
                              o_sb[:, 0:cbt].rearrange("p (b t) -> p b t",
                                                       t=T))

        for ch in range(NCHUNK):
            front(ch)
            if ch > 0:
                back(ch - 1)
            back_pre(ch)
            if ch > 1:
                back_tail(ch - 2)
        back(NCHUNK - 1)
        back_tail(NCHUNK - 2)
        back_tail(NCHUNK - 1)

    nc.finalize()
    return nc


def _prep(inputs):
    f = lambda k: np.ascontiguousarray(np.asarray(inputs[k], np.float32))
    bf = lambda a: np.ascontiguousarray(np.asarray(a, ml_dtypes.bfloat16))
    x = f("x")
    u_all = x.transpose(0, 2, 1, 3).reshape(B * N, T, C)
    u_pad = np.zeros((BSEQ, T, C), np.float32)
    u_pad[:B * N] = u_all
    xin = [bf(u_pad[i * BC:(i + 1) * BC].transpose(2, 0, 1))
           for i in range(NCORES)]

    A = -np.exp(f("A_log"))
    Ab = -np.exp(f("A_b_log"))
    assert np.allclose(A, A[0:1], rtol=1e-5), "A must be d-independent"
    assert np.allclose(Ab, A, rtol=1e-5), "A_b must equal A"
    a_pow = [float(v) for v in A[0]]

    w_in_t = f("in_proj_w").T                      # [C, 2*DI]
    w_in_x = w_in_t[:, :DI]                        # [C, DI]
    cw = np.stack([f("conv_w")[:, 0, :], f("conv_w_b")[:, 0, :]])  # [2,DI,4]
    # wconv[c, br, ti, k, d] = w_in_x[c, ti*128+d] * cw[br, ti*128+d, k]
    wconv = np.einsum('cd,bdk->bkcd', w_in_x, cw)  # [2, 4, C, DI]
    wconv = wconv.reshape(2, 4, C, 2, 128).transpose(2, 0, 3, 1, 4)
    cb = np.stack([f("conv_b"), f("conv_b_b")])[..., None]         # [2,DI,1]
    xw_ro = np.concatenate([f("xproj_w")[RK:], f("xproj_w")[:RK]])
    xw_ro_b = np.concatenate([f("xproj_w_b")[RK:], f("xproj_w_b")[:RK]])
    xwm = np.stack([xw_ro, xw_ro_b]).transpose(0, 2, 1)
    dtwm = np.stack([f("dtproj_w"), f("dtproj_w_b")]).transpose(0, 2, 1)
    dtbm = np.stack([f("dtproj_b"), f("dtproj_b_b")])[..., None]
    shared = {
        "w_z": bf(w_in_t[:, DI:]),
        "wconv": bf(wconv),
        "convb": np.ascontiguousarray(
            cb.reshape(2, 2, 128, 1).transpose(2, 0, 1, 3)),
        "xw": bf(xwm.reshape(2, 2, 128, 40).transpose(2, 0, 1, 3)),
        "dtw": bf(dtwm.transpose(1, 0, 2)),                        # [8,2,256]
        "dtb": np.ascontiguousarray(
            dtbm.reshape(2, 2, 128, 1).transpose(2, 0, 1, 3)),
        "dpc": np.ascontiguousarray(
            np.stack([f("Dp"), f("Dp_b")])[..., None]
            .reshape(2, 2, 128, 1).transpose(2, 0, 1, 3)),
        "wout": bf(
            f("out_proj_w").T.reshape(2, 128, 128).transpose(1, 0, 2)),
        "ln1g": f("ln1_g").reshape(C, 1),
        "ln1b": f("ln1_b").reshape(C, 1),
        "ln2g": f("ln2_g").reshape(C, 1),
        "ln2b": f("ln2_b").reshape(C, 1),
    }
    return xin, shared, a_pow


def _unshard(core_outs):
    y = np.stack(core_outs)                       # [8, C, BC, T]
    y = y.transpose(0, 2, 3, 1).reshape(BSEQ, T, C)[:B * N]
    return np.ascontiguousarray(
        y.reshape(B, N, T, C).transpose(0, 2, 1, 3))


_CACHE = {}


def kernel(_trace=False, **inputs):
    xin, shared, a_pow = _prep(inputs)
    if "prog" not in _CACHE:
        lt = (bool(np.all(inputs["ln1_g"] == 1) and np.all(inputs["ln1_b"] == 0)),
              bool(np.all(inputs["ln2_g"] == 1) and np.all(inputs["ln2_b"] == 0)))
        _CACHE["prog"] = build_program(a_pow, ln_trivial=lt)
    nc = _CACHE["prog"]
    in_maps = [dict(shared, xin=xin[i]) for i in range(NCORES)]
    res = run_bass_kernel_spmd(nc, in_maps, core_ids=list(range(NCORES)),
                               trace=_trace)
    out = _unshard([r["out"] for r in res.results])
    if _trace:
        kernel.last_results = res
    return out

